# revision 15
# baseline (speedup 1.0000x reference)
"""Trainium2 Bass kernel for 2-layer GAT (nn_GAT_72619307041134).

Strategy (dst-sharded edge parallelism, 8 cores SPMD):
- Nodes sharded into 8 contiguous ranges of 6250; edges sorted by dst and
  sharded by dst range, so each core owns ALL edges of its dst nodes and the
  segment softmax + aggregation need no cross-core reduction.
- Self-loops are NOT in the edge stream; their contribution is folded into
  the window epilogues (per-node dense math), saving ~6% of gather traffic.
- Per layer, a per-node gather table lives in DRAM:
    table1[n] = [al_src (8 bf16) | h1 (128 fp8)] 256B fp8 rows
    table2[n] = [al2_src (1 f32 = 2 slots) | h2 (64 bf16) | pad] 256B rows
  The dst-side attention logits (al_d) are NOT gathered per edge: within a
  128-node dst window they are broadcast to edges with a small PE matmul
  alde[e,h] = sum_j S0T[j,e] * al_d[j,h], where S0T is the transposed
  edge->node indicator, host-baked as exact fp8 0/1 and streamed from DRAM
  on the (otherwise idle) HWDGE path.
- Edges are processed in 128-edge chunks (16 chunks = 1 super-chunk):
  dma_gather fetches the src rows (int16 indices; srcs >= SPLIT gather
  from a shifted view; A/B-pure chunks, A-first in each super-chunk),
  each call round-robined over all 4 SWDGE queues (the drain of these
  random 256-512B HBM reads is the kernel's bottleneck).
  DVE computes ee = exp(leaky_relu(al_s+al_d)) and msg = [ee*h | ee] (bf16);
  an indicator S0[e,j] = (dst_rel[e]==j) turns the per-window segment-sum
  into PE matmuls accumulating in PSUM.  Window epilogue adds the self-loop
  term and divides by the softmax denominator (segment-max shift skipped;
  logits are bounded by ~4).
- Pad edges have dst_rel=-1 (S0/S0T column zero) and gather row 0.
- Between layers one AllGather shares each core's table2 shard.
- Host preprocessing only sorts/pads/permutes integer indices.
"""

from contextlib import ExitStack

import numpy as np

# ---------------------------------------------------------------------------
# config
# ---------------------------------------------------------------------------


class Cfg:
    def __init__(self, N=50000, E=800000, NCORES=8, WIN=128, CHUNK=128, SC=16,
                 SPLIT=32768):
        self.N = N
        self.E = E
        self.NCORES = NCORES
        self.NSH = N // NCORES
        self.WIN = WIN
        self.NWIN = (self.NSH + WIN - 1) // WIN
        self.CHUNK = CHUNK
        self.SC = SC
        self.SPLIT = SPLIT        # src < SPLIT -> table A view, else B view
        self.HEADS = 8
        self.HID = 16
        self.OUT_C = 64
        self.R1 = 256             # bf16 elems/row: [al_s 8 f32 = 16 slots | h1 128 bf16 | pad]
        self.H1OFF = 16
        self.R2 = 128             # bf16 elems/row: [al2_s 2 slots | h2 64 | pad]
        self.H2OFF = 2
        self.MSG1 = 128 + 8
        self.MSG2 = 64 + 1
        self.NQUEUES = 4


FULL = Cfg()

# ---------------------------------------------------------------------------
# host-side edge preprocessing (indices only)
# ---------------------------------------------------------------------------


def prep_edges(edge_index: np.ndarray, cfg: Cfg):
    """Builds the SPMD-common chunk schedule and per-core index arrays.

    meta: cmap (slot -> (window, first, last)), nA (A-chunks per super-chunk,
    A-first slot order), TC, NG.
    per_core[c]: int16 'src_idx' [16, EPC/16] (wrapped: edge i of the
    slot-ordered stream at [i%16, i//16]), f32 'dst_rel' [128, TC]
    (rel[e, slot] for the edge at (slot, partition e), -1 for pads),
    int16 's0t_bits' [128, TC*8] (bit (j, slot, e) = (rel[slot,e]==j),
    packed 16 edges/word along e).
    """
    N, NSH, WIN, CHUNK, SC = cfg.N, cfg.NSH, cfg.WIN, cfg.CHUNK, cfg.SC
    src = edge_index[0].astype(np.int64)
    dst = edge_index[1].astype(np.int64)
    order = np.argsort(dst, kind="stable")
    src = src[order]
    dst = dst[order]
    isA = src < cfg.SPLIT
    core_of = dst // NSH
    w_of = (dst % NSH) // WIN
    cntA = np.zeros((cfg.NCORES, cfg.NWIN), np.int64)
    cntB = np.zeros((cfg.NCORES, cfg.NWIN), np.int64)
    np.add.at(cntA, (core_of[isA], w_of[isA]), 1)
    np.add.at(cntB, (core_of[~isA], w_of[~isA]), 1)
    cpwA = np.ceil(cntA.max(0) / CHUNK).astype(np.int64)
    cpwB = np.ceil(cntB.max(0) / CHUNK).astype(np.int64)
    tc = int((cpwA + cpwB).sum())
    cpwA[-1] += (-tc) % SC
    TC = int((cpwA + cpwB).sum())
    NG = TC // SC

    # global chunk list: window-major, A then B inside a window
    glist = []  # (window, is_A)
    for w in range(cfg.NWIN):
        glist += [(w, True)] * int(cpwA[w]) + [(w, False)] * int(cpwB[w])
    # per-super-chunk stable reorder: A-chunks first
    slot_of = []
    nA = []
    for g in range(NG):
        blk = list(range(g * SC, (g + 1) * SC))
        a = [i for i in blk if glist[i][1]]
        b = [i for i in blk if not glist[i][1]]
        nA.append(len(a))
        slot_of += a + b
    lastslot = {}
    for s, gi in enumerate(slot_of):
        lastslot[glist[gi][0]] = s
    cmap = []
    seen = set()
    inflight = 0
    max_inflight = 0
    for s, gi in enumerate(slot_of):
        w = glist[gi][0]
        first = w not in seen
        seen.add(w)
        last = lastslot[w] == s
        if first:
            inflight += 1
            max_inflight = max(max_inflight, inflight)
        cmap.append((w, first, last))
        if last:
            inflight -= 1

    starts = np.searchsorted(
        dst, (np.arange(0, N, NSH)[:, None] + np.arange(0, NSH, WIN)[None, :]))
    flat_starts = list(starts.ravel()) + [len(dst)]
    per_core = []
    for c in range(cfg.NCORES):
        s_by_g = np.zeros((len(glist), CHUNK), np.int64)
        r_by_g = np.full((len(glist), CHUNK), -1, np.int64)
        gi = 0
        for w in range(cfg.NWIN):
            i = c * cfg.NWIN + w
            s0, s1 = flat_starts[i], flat_starts[i + 1]
            sl = src[s0:s1]
            dl = dst[s0:s1]
            il = isA[s0:s1]
            for grp in (True, False):
                m = il == grp
                se = sl[m]
                de = dl[m]
                nch = int(cpwA[w]) if grp else int(cpwB[w])
                buf_s = np.zeros(nch * CHUNK, np.int64)
                buf_r = np.full(nch * CHUNK, -1, np.int64)
                buf_s[:len(se)] = se - (0 if grp else cfg.SPLIT)
                buf_r[:len(se)] = de - (c * NSH + w * WIN)
                s_by_g[gi:gi + nch] = buf_s.reshape(nch, CHUNK)
                r_by_g[gi:gi + nch] = buf_r.reshape(nch, CHUNK)
                gi += nch
        svals = s_by_g[slot_of]   # [TC, 128] slot-ordered
        rvals = r_by_g[slot_of]

        def wrap(vals):
            st = vals.reshape(-1)   # stream i = slot*128 + p
            n = st.shape[0]
            out = np.zeros((16, n // 16), np.int16)
            idx = np.arange(n)
            out[idx % 16, idx // 16] = st
            return np.tile(out, (8, 1))

        # host-baked transposed one-hot (fp8 e4m3 bit pattern: 1.0 = 0x38)
        ONE = np.uint8(0x38)
        jj = np.arange(128, dtype=np.int64)
        # s0t[j, slot*128+e] = (rvals[slot, e] == j)
        s0t = (rvals[None, :, :] == jj[:, None, None]).astype(np.uint8) * ONE
        per_core.append(dict(
            src_idx=wrap(svals),
            rel_bf=np.ascontiguousarray(rvals.T).astype(np.float32),
            s0t_bf=np.ascontiguousarray(s0t.reshape(128, TC * 128)),
        ))
    meta = dict(cmap=cmap, nA=nA, TC=TC, NG=NG, max_inflight=max_inflight)
    return meta, per_core


def host_tensors(inputs, cfg: Cfg):
    x = np.ascontiguousarray(inputs["x"], np.float32)
    W1 = np.ascontiguousarray(inputs["W1"], np.float32)
    a1s = np.asarray(inputs["a1_src"], np.float32)
    a1d = np.asarray(inputs["a1_dst"], np.float32)
    W2 = np.ascontiguousarray(inputs["W2"], np.float32)
    a2s = np.asarray(inputs["a2_src"], np.float32).reshape(1, -1)
    a2d = np.asarray(inputs["a2_dst"], np.float32).reshape(1, -1)
    b1 = np.asarray(inputs["b1"], np.float32)
    b2 = np.asarray(inputs["b2"], np.float32)
    H, HID = cfg.HEADS, cfg.HID
    A1 = np.zeros((H * HID, 2 * H), np.float32)
    for h in range(H):
        A1[h * HID:(h + 1) * HID, h] = a1s[h]
        A1[h * HID:(h + 1) * HID, H + h] = a1d[h]
    RHS2 = np.concatenate([W2 @ a2s.T, W2 @ a2d.T, W2], 1).astype(np.float32)
    shared = dict(W1=W1, A1=A1, RHS2=RHS2,
                  b1row=b1.reshape(1, -1),
                  b2row=np.concatenate([np.zeros(2, np.float32), b2]).reshape(1, -1))
    xT = [np.ascontiguousarray(x[c * cfg.NSH:(c + 1) * cfg.NSH].T)
          for c in range(cfg.NCORES)]
    has_bias = bool(np.any(b1) or np.any(b2))
    return shared, xT, has_bias


# ---------------------------------------------------------------------------
# device kernel emission
# ---------------------------------------------------------------------------


def _ap(base, free_dims, extra_off=0):
    """Replace the free dims of a [P, ...] AP (keep partition dim)."""
    import concourse.bass as bass

    return bass.AP(base.tensor, base.offset + extra_off,
                   [list(base.ap[0])] + [list(d) for d in free_dims])


def emit_gat(tc, out_ap, ins, meta, cfg: Cfg, has_bias=False):
    import concourse.bass as bass  # noqa: F401
    from concourse import mybir

    nc = tc.nc
    f32 = mybir.dt.float32
    bf16 = mybir.dt.bfloat16
    fp8 = mybir.dt.float8e4
    i16 = mybir.dt.int16
    i32 = mybir.dt.int32
    AF = mybir.ActivationFunctionType
    OP = mybir.AluOpType
    N, NSH, WIN, NWIN, SC = cfg.N, cfg.NSH, cfg.WIN, cfg.NWIN, cfg.SC
    TC, NG = meta["TC"], meta["NG"]
    cmap, nA = meta["cmap"], meta["nA"]
    NQ = getattr(cfg, "NQUEUES", 1)
    R1, R2 = cfg.R1, cfg.R2
    M1, M2 = cfg.MSG1, cfg.MSG2
    NIW = TC * cfg.CHUNK // 16

    ctx = ExitStack()
    with ctx:
        dram = ctx.enter_context(tc.tile_pool(name="dram", bufs=1, space="DRAM"))
        consts = ctx.enter_context(tc.tile_pool(name="consts", bufs=1))

        t1shard = dram.tile([NSH, R1], bf16)
        t1full = dram.tile([N, R1], bf16, addr_space="Shared")
        t2shard = dram.tile([NSH, R2], bf16)
        t2full = dram.tile([N, R2], bf16, addr_space="Shared")

        # ------- constants into SBUF -------
        W1_sb = consts.tile([128, 128], f32)
        A1_sb = consts.tile([128, 16], f32)
        RHS2_sb = consts.tile([128, 66], f32)
        nc.sync.dma_start(W1_sb[:], ins["W1"][:])
        nc.sync.dma_start(A1_sb[:], ins["A1"][:])
        nc.sync.dma_start(RHS2_sb[:], ins["RHS2"][:])
        src_sb = consts.tile([128, NIW], i16)
        nc.sync.dma_start(src_sb[:], ins["src_idx"][:])
        relf_sb = consts.tile([128, TC], f32)
        rel_sb = consts.tile([128, TC], bf16)
        nc.sync.dma_start(relf_sb[:], ins["rel_bf"][:])
        if has_bias:
            b1_sb = consts.tile([1, 128], f32)
            b2_sb = consts.tile([1, 66], f32)
            ones_sb = consts.tile([1, 128], f32)
            nc.sync.dma_start(b1_sb[:], ins["b1row"][:])
            nc.sync.dma_start(b2_sb[:], ins["b2row"][:])
            nc.gpsimd.memset(ones_sb[:], 1.0)

        # identity + iota + bitmask
        iota_i = consts.tile([128, 128], i32)
        icol = consts.tile([128, 1], i32)
        id_sb = consts.tile([128, 128], f32)
        iota_f = consts.tile([128, 128], f32)
        icol_f = consts.tile([128, 1], f32)
        nc.gpsimd.iota(iota_i[:], pattern=[[1, 128]], base=0, channel_multiplier=0)
        nc.gpsimd.iota(icol[:], pattern=[[1, 1]], base=0, channel_multiplier=1)
        nc.vector.tensor_copy(iota_f[:], iota_i[:])
        nc.vector.tensor_copy(icol_f[:], icol[:])
        nc.vector.tensor_scalar(id_sb[:], iota_f[:], icol_f[:], None, OP.is_equal)
        iota_bf = consts.tile([128, 128], bf16)
        nc.vector.tensor_copy(iota_bf[:], iota_f[:])
        nc.vector.tensor_copy(rel_sb[:], relf_sb[:])

        # persistent per-node state (per window layout)
        als_sb = consts.tile([128, NWIN * 16], f32)     # [al_s 8 | al_d 8]
        aldw_bf = consts.tile([128, NWIN * 8], bf16)    # al_d (L1)
        al2_sb = consts.tile([128, NWIN * 2], f32)      # [al2_s, al2_d]
        al2_bf = consts.tile([128, NWIN], bf16)         # al2_d
        h1w_sb = consts.tile([128, NWIN * 128], bf16)   # h1 rows per window
        h2w_sb = consts.tile([128, NWIN * 64], bf16)    # h2 rows per window
        nc.vector.memset(aldw_bf[:], 0.0)
        nc.vector.memset(al2_bf[:], 0.0)

        # ------- stage A: h1, al, table1 -------
        with tc.tile_pool(name="stageA", bufs=1) as sa, \
             tc.tile_pool(name="stageApsum", bufs=2, space="PSUM") as sap, \
             tc.tile_pool(name="rows", bufs=3) as rows:
            xT_sb = sa.tile([128, NSH], f32)
            nc.sync.dma_start(xT_sb[:], ins["xT"][:])
            h1T_sb = sa.tile([128, NSH], f32)
            al_sb = sa.tile([16, NSH], f32)
            nt = (NSH + 511) // 512
            for j in range(nt):
                w0 = j * 512
                w1 = min(NSH, w0 + 512)
                ph = sap.tile([128, 512], f32, tag="ph")
                nc.tensor.matmul(ph[:, : w1 - w0], W1_sb[:], xT_sb[:, w0:w1],
                                 start=True, stop=True)
                nc.vector.tensor_copy(h1T_sb[:, w0:w1], ph[:, : w1 - w0])
            for j in range(nt):
                w0 = j * 512
                w1 = min(NSH, w0 + 512)
                pa = sap.tile([16, 512], f32, tag="pa")
                nc.tensor.matmul(pa[:, : w1 - w0], A1_sb[:], h1T_sb[:, w0:w1],
                                 start=True, stop=True)
                nc.vector.tensor_copy(al_sb[:, w0:w1], pa[:, : w1 - w0])

            for w in range(NWIN):
                w0 = w * WIN
                wn = min(WIN, NSH - w0)
                hp = sap.tile([128, 128], f32, tag="hp")
                if has_bias:
                    nc.tensor.matmul(hp[:wn, :], xT_sb[:, w0:w0 + wn], W1_sb[:],
                                     start=True, stop=False)
                    nc.tensor.matmul(hp[:wn, :], ones_sb[0:1, :wn], b1_sb[:],
                                     start=False, stop=True)
                else:
                    nc.tensor.matmul(hp[:wn, :], xT_sb[:, w0:w0 + wn], W1_sb[:],
                                     start=True, stop=True)
                at = sap.tile([128, 16], f32, tag="at")
                nc.tensor.transpose(at[:wn, :], al_sb[:, w0:w0 + wn], id_sb[:16, :16])
                rowt = rows.tile([128, R1], bf16, tag="rowt")
                nc.vector.tensor_copy(rowt[:wn, 0:16].bitcast(f32), at[:wn, 0:8])
                nc.scalar.copy(rowt[:wn, 16:144], hp[:wn, :])
                nc.vector.memset(rowt[:wn, 144:R1], 0.0)
                nc.sync.dma_start(t1shard.opt()[w0:w0 + wn, :], rowt[:wn, :])
                nc.vector.tensor_copy(als_sb[:wn, w * 16:(w + 1) * 16], at[:wn, :])
                nc.vector.tensor_copy(aldw_bf[:wn, w * 8:(w + 1) * 8],
                                      at[:wn, 8:16])
                nc.vector.tensor_copy(h1w_sb[:wn, w * 128:(w + 1) * 128],
                                      hp[:wn, :])

        from concourse import library_config

        nc.gpsimd.load_library(library_config.mlp)

        nc.gpsimd.collective_compute(
            "AllGather", mybir.AluOpType.bypass,
            replica_groups=[list(range(cfg.NCORES))],
            ins=[t1shard.opt()], outs=[t1full.opt()],
        )

        # ------- edge layers -------
        def edge_layer(table_full, row, rdt, alst, hoff, nh, chper, msgc,
                       alw_bf, epilogue):
            with tc.tile_pool(name="edges", bufs=5) as epool, \
                 tc.tile_pool(name="emsg", bufs=3) as mp, \
                 tc.tile_pool(name="epsum", bufs=meta["max_inflight"] + 1,
                              space="PSUM") as pp, \
                 tc.tile_pool(name="aldpsum", bufs=2, space="PSUM") as pae, \
                 tc.tile_pool(name="esmall", bufs=4) as spool:
                pw_by_w = {}
                qctr = [0]
                for g in range(NG):
                    na = nA[g]
                    gb = epool.tile([128, SC * row], rdt, tag="gb")
                    gb3 = gb[:].rearrange("p (k e) -> p k e", k=SC)
                    c0 = g * SC * 8
                    for grp in range(2):
                        nch = na if grp == 0 else SC - na
                        if nch == 0:
                            continue
                        ksl = slice(0, na) if grp == 0 else slice(na, SC)
                        csl = (slice(c0, c0 + na * 8) if grp == 0
                               else slice(c0 + na * 8, c0 + SC * 8))
                        tbl = (table_full.opt() if grp == 0
                               else table_full.opt()[cfg.SPLIT:N, :])
                        nc.gpsimd.dma_gather(
                            gb3[:, ksl, :], tbl, src_sb[:, csl],
                            num_idxs=nch * 128, num_idxs_reg=nch * 128,
                            elem_size=row, single_packet=False,
                            queue_num=qctr[0] % NQ,
                        )
                        qctr[0] += 1
                    # S0T [j, e] streamed from DRAM (host-baked)
                    s0t = epool.tile([128, SC * 128], fp8, tag="s0t")
                    nc.sync.dma_start(
                        s0t[:], ins["s0t_bf"][:, g * SC * 128:(g + 1) * SC * 128])
                    # al_d per edge via PE: alde[e, h] = sum_j S0T[j,e] ald[j,h]
                    pa = pae.tile([128, SC * nh], f32, tag="pa", name="pa")
                    for k in range(SC):
                        w = cmap[g * SC + k][0]
                        nc.tensor.matmul(
                            pa[:, k * nh:(k + 1) * nh],
                            s0t[:, k * 128:(k + 1) * 128],
                            alw_bf[:, w * nh:(w + 1) * nh],
                            start=True, stop=True)
                    lg = spool.tile([128, SC * nh], f32, tag="lg")
                    nc.vector.tensor_tensor(
                        _ap(lg[:], [[nh, SC], [1, nh]]),
                        _ap(gb[:].bitcast(f32), [[alst, SC], [1, nh]]),
                        _ap(pa[:], [[nh, SC], [1, nh]]),
                        OP.add,
                    )
                    lr = spool.tile([128, SC * nh], f32, tag="lr")
                    nc.vector.scalar_tensor_tensor(
                        lr[:], lg[:], 0.2, lg[:], OP.mult, OP.max)
                    ee = spool.tile([128, SC * nh], bf16, tag="ee")
                    nc.scalar.activation(ee[:], lr[:], AF.Exp)
                    msg = mp.tile([128, SC * msgc], bf16, tag="msg")
                    nc.vector.tensor_tensor(
                        _ap(msg[:], [[msgc, SC], [chper, nh], [1, chper]]),
                        _ap(gb[:], [[row, SC], [chper, nh], [1, chper]], hoff),
                        _ap(ee[:], [[nh, SC], [1, nh], [0, chper]]),
                        OP.mult,
                    )
                    nc.scalar.copy(
                        _ap(msg[:], [[msgc, SC], [1, nh]], msgc - nh),
                        _ap(ee[:], [[nh, SC], [1, nh]]),
                    )
                    s0 = epool.tile([128, SC * 128], bf16, tag="s0")
                    nc.vector.tensor_tensor(
                        _ap(s0[:], [[128, SC], [1, 128]]),
                        _ap(iota_bf[:], [[0, SC], [1, 128]]),
                        _ap(rel_sb[:, g * SC:(g + 1) * SC], [[1, SC], [0, 128]]),
                        OP.is_equal,
                    )
                    for k in range(SC):
                        kk = g * SC + k
                        w, first, last = cmap[kk]
                        if first:
                            pw_by_w[w] = pp.tile([128, msgc], f32, tag="pw", name="pw")
                        pw = pw_by_w[w]
                        nc.tensor.matmul(
                            pw[:], s0[:, k * 128:(k + 1) * 128],
                            msg[:, k * msgc:(k + 1) * msgc],
                            start=first, stop=last,
                        )
                        if last:
                            epilogue(w, pw_by_w.pop(w))

        # ---- L1 ----
        with tc.tile_pool(name="epi1", bufs=2) as hq, \
             tc.tile_pool(name="epi1p", bufs=1, space="PSUM") as hpp:
            def epi1(w, pw):
                w0 = w * WIN
                wn = min(WIN, NSH - w0)
                # self-loop term
                lgs = hq.tile([128, 8], f32, tag="lgs")
                nc.vector.tensor_tensor(lgs[:], als_sb[:, w * 16:w * 16 + 8],
                                        als_sb[:, w * 16 + 8:w * 16 + 16], OP.add)
                ees = hq.tile([128, 8], f32, tag="ees")
                nc.vector.scalar_tensor_tensor(
                    ees[:], lgs[:], 0.2, lgs[:], OP.mult, OP.max)
                nc.scalar.activation(ees[:], ees[:], AF.Exp)
                dn = hq.tile([128, 8], f32, tag="dn")
                nc.vector.scalar_tensor_tensor(
                    dn[:], pw[:, 128:136], 1e-16, ees[:], OP.add, OP.add)
                rcp = hq.tile([128, 8], f32, tag="rcp")
                nc.vector.reciprocal(rcp[:], dn[:])
                uu = hq.tile([128, 128], f32, tag="uu")
                nc.vector.tensor_tensor(
                    _ap(uu[:], [[16, 8], [1, 16]]),
                    _ap(h1w_sb[:, w * 128:(w + 1) * 128], [[16, 8], [1, 16]]),
                    _ap(ees[:], [[1, 8], [0, 16]]),
                    OP.mult,
                )
                nc.vector.tensor_tensor(uu[:], uu[:], pw[:, 0:128], OP.add)
                hb = hq.tile([128, 128], f32, tag="hb")
                nc.vector.tensor_tensor(
                    _ap(hb[:], [[16, 8], [1, 16]]),
                    _ap(uu[:], [[16, 8], [1, 16]]),
                    _ap(rcp[:], [[1, 8], [0, 16]]),
                    OP.mult,
                )
                nc.scalar.activation(hb[:], hb[:], AF.Relu)
                tp = hpp.tile([128, 128], f32, tag="tp")
                nc.tensor.transpose(tp[:], hb[:], id_sb[:])
                tH = hq.tile([128, 128], f32, tag="tH")
                nc.vector.tensor_copy(tH[:], tp[:])
                p2 = hpp.tile([128, 66], f32, tag="p2")
                if has_bias:
                    nc.tensor.matmul(p2[:], tH[:], RHS2_sb[:], start=True, stop=False)
                    nc.tensor.matmul(p2[:], ones_sb[0:1, :128], b2_sb[:],
                                     start=False, stop=True)
                else:
                    nc.tensor.matmul(p2[:], tH[:], RHS2_sb[:], start=True, stop=True)
                t2b = hq.tile([128, R2], bf16, tag="t2b")
                nc.vector.tensor_copy(t2b[:wn, 0:2].bitcast(f32), p2[:wn, 0:1])
                nc.scalar.copy(t2b[:wn, 2:66], p2[:wn, 2:66])
                nc.vector.memset(t2b[:wn, 66:R2], 0.0)
                nc.sync.dma_start(t2shard.opt()[w0:w0 + wn, :], t2b[:wn, :])
                nc.vector.tensor_copy(al2_sb[:wn, w * 2:(w + 1) * 2], p2[:wn, 0:2])
                nc.vector.tensor_copy(al2_bf[:wn, w:w + 1], p2[:wn, 1:2])
                nc.vector.tensor_copy(h2w_sb[:wn, w * 64:(w + 1) * 64],
                                      p2[:wn, 2:66])

            edge_layer(t1full, cfg.R1, mybir.dt.bfloat16, 128, cfg.H1OFF,
                       8, 16, M1, aldw_bf, epi1)

        nc.gpsimd.collective_compute(
            "AllGather", mybir.AluOpType.bypass,
            replica_groups=[list(range(cfg.NCORES))],
            ins=[t2shard.opt()], outs=[t2full.opt()],
        )

        # ---- L2 ----
        with tc.tile_pool(name="epi2", bufs=2) as oq:
            def epi2(w, pw):
                w0 = w * WIN
                wn = min(WIN, NSH - w0)
                lg2 = oq.tile([128, 1], f32, tag="lg2")
                nc.vector.tensor_tensor(lg2[:], al2_sb[:, w * 2:w * 2 + 1],
                                        al2_sb[:, w * 2 + 1:w * 2 + 2], OP.add)
                ee2 = oq.tile([128, 1], f32, tag="ee2")
                nc.vector.scalar_tensor_tensor(
                    ee2[:], lg2[:], 0.2, lg2[:], OP.mult, OP.max)
                nc.scalar.activation(ee2[:], ee2[:], AF.Exp)
                dn2 = oq.tile([128, 1], f32, tag="dn2")
                nc.vector.scalar_tensor_tensor(
                    dn2[:], pw[:, 64:65], 1e-16, ee2[:], OP.add, OP.add)
                rcp2 = oq.tile([128, 1], f32, tag="rcp2")
                nc.vector.reciprocal(rcp2[:], dn2[:])
                ms2 = oq.tile([128, 64], f32, tag="ms2")
                nc.vector.tensor_tensor(
                    ms2[:], h2w_sb[:, w * 64:(w + 1) * 64],
                    _ap(ee2[:], [[0, 64]]), OP.mult)
                nc.vector.tensor_tensor(ms2[:], ms2[:], pw[:, 0:64], OP.add)
                ob = oq.tile([128, 64], f32, tag="ob")
                nc.vector.tensor_tensor(ob[:], ms2[:], _ap(rcp2[:], [[0, 64]]),
                                        OP.mult)
                nc.sync.dma_start(out_ap[w0:w0 + wn, :], ob[:wn, :])

            edge_layer(t2full, cfg.R2, mybir.dt.bfloat16, 64, cfg.H2OFF,
                       1, 64, M2, al2_bf, epi2)


# ---------------------------------------------------------------------------
# SPMD build + run
# ---------------------------------------------------------------------------

_CACHE = {}


def _build(meta, cfg: Cfg, has_bias: bool):
    key = (tuple(meta["cmap"]), tuple(meta["nA"]), cfg.N, cfg.NCORES, has_bias)
    if key in _CACHE:
        return _CACHE[key]
    import concourse.tile as tile
    from concourse import bacc, mybir

    f32 = mybir.dt.float32
    i16 = mybir.dt.int16
    TC = meta["TC"]
    NIW = TC * cfg.CHUNK // 16
    nc = bacc.Bacc("TRN2", target_bir_lowering=False, debug=False,
                   num_devices=cfg.NCORES,
                   num_swdge_queues=getattr(cfg, "NQUEUES", 1))
    ins = {}

    def di(name, shape, dt=f32):
        ins[name] = nc.dram_tensor(name, shape, dt, kind="ExternalInput").ap()

    di("xT", [128, cfg.NSH])
    di("W1", [128, 128])
    di("A1", [128, 16])
    di("RHS2", [128, 66])
    di("src_idx", [128, NIW], i16)
    di("rel_bf", [128, TC])
    di("s0t_bf", [128, TC * 128], mybir.dt.float8e4)
    if has_bias:
        di("b1row", [1, 128])
        di("b2row", [1, 66])
    out = nc.dram_tensor("out", [cfg.NSH, cfg.OUT_C], f32, kind="ExternalOutput").ap()

    with tile.TileContext(nc) as tc:
        emit_gat(tc, out, ins, meta, cfg, has_bias)
    nc.compile()
    _CACHE[key] = nc
    return nc


def kernel(**inputs) -> np.ndarray:
    out, _ = _run(inputs)
    return out


def _run(inputs, **run_kwargs):
    cfg = FULL
    inputs = {k: np.asarray(v) for k, v in inputs.items()}
    edge_index = inputs["edge_index"].astype(np.int64)
    meta, per_core = prep_edges(edge_index, cfg)
    shared, xT, has_bias = host_tensors(inputs, cfg)
    nc = _build(meta, cfg, has_bias)

    from concourse.bass_utils import run_bass_kernel_spmd

    import ml_dtypes

    in_maps = []
    for c in range(cfg.NCORES):
        m = {k: shared[k] for k in ("W1", "A1", "RHS2")}
        if has_bias:
            m["b1row"] = shared["b1row"]
            m["b2row"] = shared["b2row"]
        m["xT"] = xT[c]
        pc = dict(per_core[c])
        pc["s0t_bf"] = pc["s0t_bf"].view(ml_dtypes.float8_e4m3fn)
        m.update(pc)
        in_maps.append(m)
    res = run_bass_kernel_spmd(nc, in_maps, core_ids=list(range(cfg.NCORES)),
                               **run_kwargs)
    out = np.concatenate([res.results[c]["out"] for c in range(cfg.NCORES)], 0)
    return out.astype(np.float32), res


# revision 16
# speedup vs baseline: 1.0058x; 1.0058x over previous
"""Trainium2 Bass kernel for 2-layer GAT (nn_GAT_72619307041134).

Strategy (dst-sharded edge parallelism, 8 cores SPMD):
- Nodes sharded into 8 contiguous ranges of 6250; edges sorted by dst and
  sharded by dst range, so each core owns ALL edges of its dst nodes and the
  segment softmax + aggregation need no cross-core reduction.
- Self-loops are NOT in the edge stream; their contribution is folded into
  the window epilogues (per-node dense math), saving ~6% of gather traffic.
- Per layer, a per-node gather table lives in DRAM:
    table1[n] = [al_src (8 bf16) | h1 (128 fp8)] 256B fp8 rows
    table2[n] = [al2_src (1 f32 = 2 slots) | h2 (64 bf16) | pad] 256B rows
  The dst-side attention logits (al_d) are NOT gathered per edge: within a
  128-node dst window they are broadcast to edges with a small PE matmul
  alde[e,h] = sum_j S0T[j,e] * al_d[j,h], where S0T is the transposed
  edge->node indicator, host-baked as exact fp8 0/1 and streamed from DRAM
  on the (otherwise idle) HWDGE path.
- Edges are processed in 128-edge chunks (16 chunks = 1 super-chunk):
  dma_gather fetches the src rows (int16 indices; srcs >= SPLIT gather
  from a shifted view; A/B-pure chunks, A-first in each super-chunk),
  each call round-robined over all 4 SWDGE queues (the drain of these
  random 256-512B HBM reads is the kernel's bottleneck).
  DVE computes ee = exp(leaky_relu(al_s+al_d)) and msg = [ee*h | ee] (bf16);
  an indicator S0[e,j] = (dst_rel[e]==j) turns the per-window segment-sum
  into PE matmuls accumulating in PSUM.  Window epilogue adds the self-loop
  term and divides by the softmax denominator (segment-max shift skipped;
  logits are bounded by ~4).
- Pad edges have dst_rel=-1 (S0/S0T column zero) and gather row 0.
- Between layers one AllGather shares each core's table2 shard.
- Host preprocessing only sorts/pads/permutes integer indices.
"""

from contextlib import ExitStack

import numpy as np

# ---------------------------------------------------------------------------
# config
# ---------------------------------------------------------------------------


class Cfg:
    def __init__(self, N=50000, E=800000, NCORES=8, WIN=128, CHUNK=128, SC=16,
                 SPLIT=32768):
        self.N = N
        self.E = E
        self.NCORES = NCORES
        self.NSH = N // NCORES
        self.WIN = WIN
        self.NWIN = (self.NSH + WIN - 1) // WIN
        self.CHUNK = CHUNK
        self.SC = SC
        self.SPLIT = SPLIT        # src < SPLIT -> table A view, else B view
        self.HEADS = 8
        self.HID = 16
        self.OUT_C = 64
        self.R1 = 256             # bf16 elems/row: [al_s 8 f32 = 16 slots | h1 128 bf16 | pad]
        self.H1OFF = 16
        self.R2 = 128             # bf16 elems/row: [al2_s 2 slots | h2 64 | pad]
        self.H2OFF = 2
        self.MSG1 = 128 + 8
        self.MSG2 = 64 + 1
        self.NQUEUES = 4


FULL = Cfg()

# ---------------------------------------------------------------------------
# host-side edge preprocessing (indices only)
# ---------------------------------------------------------------------------


def prep_edges(edge_index: np.ndarray, cfg: Cfg):
    """Builds the SPMD-common chunk schedule and per-core index arrays.

    meta: cmap (slot -> (window, first, last)), nA (A-chunks per super-chunk,
    A-first slot order), TC, NG.
    per_core[c]: int16 'src_idx' [16, EPC/16] (wrapped: edge i of the
    slot-ordered stream at [i%16, i//16]), f32 'dst_rel' [128, TC]
    (rel[e, slot] for the edge at (slot, partition e), -1 for pads),
    int16 's0t_bits' [128, TC*8] (bit (j, slot, e) = (rel[slot,e]==j),
    packed 16 edges/word along e).
    """
    N, NSH, WIN, CHUNK, SC = cfg.N, cfg.NSH, cfg.WIN, cfg.CHUNK, cfg.SC
    src = edge_index[0].astype(np.int64)
    dst = edge_index[1].astype(np.int64)
    order = np.argsort(dst, kind="stable")
    src = src[order]
    dst = dst[order]
    isA = src < cfg.SPLIT
    core_of = dst // NSH
    w_of = (dst % NSH) // WIN
    cntA = np.zeros((cfg.NCORES, cfg.NWIN), np.int64)
    cntB = np.zeros((cfg.NCORES, cfg.NWIN), np.int64)
    np.add.at(cntA, (core_of[isA], w_of[isA]), 1)
    np.add.at(cntB, (core_of[~isA], w_of[~isA]), 1)
    cpwA = np.ceil(cntA.max(0) / CHUNK).astype(np.int64)
    cpwB = np.ceil(cntB.max(0) / CHUNK).astype(np.int64)
    tc = int((cpwA + cpwB).sum())
    cpwA[-1] += (-tc) % SC
    TC = int((cpwA + cpwB).sum())
    NG = TC // SC

    # global chunk list: window-major, A then B inside a window
    glist = []  # (window, is_A)
    for w in range(cfg.NWIN):
        glist += [(w, True)] * int(cpwA[w]) + [(w, False)] * int(cpwB[w])
    # per-super-chunk stable reorder: A-chunks first
    slot_of = []
    nA = []
    for g in range(NG):
        blk = list(range(g * SC, (g + 1) * SC))
        a = [i for i in blk if glist[i][1]]
        b = [i for i in blk if not glist[i][1]]
        nA.append(len(a))
        slot_of += a + b
    lastslot = {}
    for s, gi in enumerate(slot_of):
        lastslot[glist[gi][0]] = s
    cmap = []
    seen = set()
    inflight = 0
    max_inflight = 0
    for s, gi in enumerate(slot_of):
        w = glist[gi][0]
        first = w not in seen
        seen.add(w)
        last = lastslot[w] == s
        if first:
            inflight += 1
            max_inflight = max(max_inflight, inflight)
        cmap.append((w, first, last))
        if last:
            inflight -= 1

    starts = np.searchsorted(
        dst, (np.arange(0, N, NSH)[:, None] + np.arange(0, NSH, WIN)[None, :]))
    flat_starts = list(starts.ravel()) + [len(dst)]
    per_core = []
    for c in range(cfg.NCORES):
        s_by_g = np.zeros((len(glist), CHUNK), np.int64)
        r_by_g = np.full((len(glist), CHUNK), -1, np.int64)
        gi = 0
        for w in range(cfg.NWIN):
            i = c * cfg.NWIN + w
            s0, s1 = flat_starts[i], flat_starts[i + 1]
            sl = src[s0:s1]
            dl = dst[s0:s1]
            il = isA[s0:s1]
            for grp in (True, False):
                m = il == grp
                se = sl[m]
                de = dl[m]
                nch = int(cpwA[w]) if grp else int(cpwB[w])
                buf_s = np.zeros(nch * CHUNK, np.int64)
                buf_r = np.full(nch * CHUNK, -1, np.int64)
                buf_s[:len(se)] = se - (0 if grp else cfg.SPLIT)
                buf_r[:len(se)] = de - (c * NSH + w * WIN)
                s_by_g[gi:gi + nch] = buf_s.reshape(nch, CHUNK)
                r_by_g[gi:gi + nch] = buf_r.reshape(nch, CHUNK)
                gi += nch
        svals = s_by_g[slot_of]   # [TC, 128] slot-ordered
        rvals = r_by_g[slot_of]

        def wrap(vals):
            st = vals.reshape(-1)   # stream i = slot*128 + p
            n = st.shape[0]
            out = np.zeros((16, n // 16), np.int16)
            idx = np.arange(n)
            out[idx % 16, idx // 16] = st
            return np.tile(out, (8, 1))

        # host-baked transposed one-hot (fp8 e4m3 bit pattern: 1.0 = 0x38)
        ONE = np.uint8(0x38)
        jj = np.arange(128, dtype=np.int64)
        # s0t[j, slot*128+e] = (rvals[slot, e] == j)
        s0t = (rvals[None, :, :] == jj[:, None, None]).astype(np.uint8) * ONE
        per_core.append(dict(
            src_idx=wrap(svals),
            rel_bf=np.ascontiguousarray(rvals.T).astype(np.float32),
            s0t_bf=np.ascontiguousarray(s0t.reshape(128, TC * 128)),
        ))
    meta = dict(cmap=cmap, nA=nA, TC=TC, NG=NG, max_inflight=max_inflight)
    return meta, per_core


def host_tensors(inputs, cfg: Cfg):
    x = np.ascontiguousarray(inputs["x"], np.float32)
    W1 = np.ascontiguousarray(inputs["W1"], np.float32)
    a1s = np.asarray(inputs["a1_src"], np.float32)
    a1d = np.asarray(inputs["a1_dst"], np.float32)
    W2 = np.ascontiguousarray(inputs["W2"], np.float32)
    a2s = np.asarray(inputs["a2_src"], np.float32).reshape(1, -1)
    a2d = np.asarray(inputs["a2_dst"], np.float32).reshape(1, -1)
    b1 = np.asarray(inputs["b1"], np.float32)
    b2 = np.asarray(inputs["b2"], np.float32)
    H, HID = cfg.HEADS, cfg.HID
    A1 = np.zeros((H * HID, 2 * H), np.float32)
    for h in range(H):
        A1[h * HID:(h + 1) * HID, h] = a1s[h]
        A1[h * HID:(h + 1) * HID, H + h] = a1d[h]
    RHS2 = np.concatenate([W2 @ a2s.T, W2 @ a2d.T, W2], 1).astype(np.float32)
    shared = dict(W1=W1, A1=A1, RHS2=RHS2,
                  b1row=b1.reshape(1, -1),
                  b2row=np.concatenate([np.zeros(2, np.float32), b2]).reshape(1, -1))
    xT = [np.ascontiguousarray(x[c * cfg.NSH:(c + 1) * cfg.NSH].T)
          for c in range(cfg.NCORES)]
    has_bias = bool(np.any(b1) or np.any(b2))
    return shared, xT, has_bias


# ---------------------------------------------------------------------------
# device kernel emission
# ---------------------------------------------------------------------------


def _ap(base, free_dims, extra_off=0):
    """Replace the free dims of a [P, ...] AP (keep partition dim)."""
    import concourse.bass as bass

    return bass.AP(base.tensor, base.offset + extra_off,
                   [list(base.ap[0])] + [list(d) for d in free_dims])


def emit_gat(tc, out_ap, ins, meta, cfg: Cfg, has_bias=False):
    import concourse.bass as bass  # noqa: F401
    from concourse import mybir

    nc = tc.nc
    f32 = mybir.dt.float32
    bf16 = mybir.dt.bfloat16
    fp8 = mybir.dt.float8e4
    i16 = mybir.dt.int16
    i32 = mybir.dt.int32
    AF = mybir.ActivationFunctionType
    OP = mybir.AluOpType
    N, NSH, WIN, NWIN, SC = cfg.N, cfg.NSH, cfg.WIN, cfg.NWIN, cfg.SC
    TC, NG = meta["TC"], meta["NG"]
    cmap, nA = meta["cmap"], meta["nA"]
    NQ = getattr(cfg, "NQUEUES", 1)
    R1, R2 = cfg.R1, cfg.R2
    M1, M2 = cfg.MSG1, cfg.MSG2
    NIW = TC * cfg.CHUNK // 16

    ctx = ExitStack()
    with ctx:
        dram = ctx.enter_context(tc.tile_pool(name="dram", bufs=1, space="DRAM"))
        consts = ctx.enter_context(tc.tile_pool(name="consts", bufs=1))

        t1shard = dram.tile([NSH, R1], bf16)
        t1full = dram.tile([N, R1], bf16, addr_space="Shared")
        t2shard = dram.tile([NSH, R2], bf16)
        t2full = dram.tile([N, R2], bf16, addr_space="Shared")

        # ------- constants into SBUF -------
        W1_sb = consts.tile([128, 128], f32)
        A1_sb = consts.tile([128, 16], f32)
        RHS2_sb = consts.tile([128, 66], f32)
        nc.sync.dma_start(W1_sb[:], ins["W1"][:])
        nc.sync.dma_start(A1_sb[:], ins["A1"][:])
        nc.sync.dma_start(RHS2_sb[:], ins["RHS2"][:])
        src_sb = consts.tile([128, NIW], i16)
        nc.sync.dma_start(src_sb[:], ins["src_idx"][:])
        relf_sb = consts.tile([128, TC], f32)
        rel_sb = consts.tile([128, TC], bf16)
        nc.sync.dma_start(relf_sb[:], ins["rel_bf"][:])
        if has_bias:
            b1_sb = consts.tile([1, 128], f32)
            b2_sb = consts.tile([1, 66], f32)
            ones_sb = consts.tile([1, 128], f32)
            nc.sync.dma_start(b1_sb[:], ins["b1row"][:])
            nc.sync.dma_start(b2_sb[:], ins["b2row"][:])
            nc.gpsimd.memset(ones_sb[:], 1.0)

        # identity + iota + bitmask
        iota_i = consts.tile([128, 128], i32)
        icol = consts.tile([128, 1], i32)
        id_sb = consts.tile([128, 128], f32)
        iota_f = consts.tile([128, 128], f32)
        icol_f = consts.tile([128, 1], f32)
        nc.gpsimd.iota(iota_i[:], pattern=[[1, 128]], base=0, channel_multiplier=0)
        nc.gpsimd.iota(icol[:], pattern=[[1, 1]], base=0, channel_multiplier=1)
        nc.vector.tensor_copy(iota_f[:], iota_i[:])
        nc.vector.tensor_copy(icol_f[:], icol[:])
        nc.vector.tensor_scalar(id_sb[:], iota_f[:], icol_f[:], None, OP.is_equal)
        iota_bf = consts.tile([128, 128], bf16)
        nc.vector.tensor_copy(iota_bf[:], iota_f[:])
        nc.vector.tensor_copy(rel_sb[:], relf_sb[:])

        # persistent per-node state (per window layout)
        als_sb = consts.tile([128, NWIN * 16], f32)     # [al_s 8 | al_d 8]
        aldw_bf = consts.tile([128, NWIN * 8], bf16)    # al_d (L1)
        al2_sb = consts.tile([128, NWIN * 2], f32)      # [al2_s, al2_d]
        al2_bf = consts.tile([128, NWIN], bf16)         # al2_d
        h1w_sb = consts.tile([128, NWIN * 128], bf16)   # h1 rows per window
        h2w_sb = consts.tile([128, NWIN * 64], bf16)    # h2 rows per window
        nc.vector.memset(aldw_bf[:], 0.0)
        nc.vector.memset(al2_bf[:], 0.0)

        # ------- stage A: h1, al, table1 -------
        with tc.tile_pool(name="stageA", bufs=1) as sa, \
             tc.tile_pool(name="stageApsum", bufs=2, space="PSUM") as sap, \
             tc.tile_pool(name="rows", bufs=3) as rows:
            xT_sb = sa.tile([128, NSH], f32)
            nc.sync.dma_start(xT_sb[:], ins["xT"][:])
            h1T_sb = sa.tile([128, NSH], f32)
            al_sb = sa.tile([16, NSH], f32)
            nt = (NSH + 511) // 512
            for j in range(nt):
                w0 = j * 512
                w1 = min(NSH, w0 + 512)
                ph = sap.tile([128, 512], f32, tag="ph")
                nc.tensor.matmul(ph[:, : w1 - w0], W1_sb[:], xT_sb[:, w0:w1],
                                 start=True, stop=True)
                nc.vector.tensor_copy(h1T_sb[:, w0:w1], ph[:, : w1 - w0])
            for j in range(nt):
                w0 = j * 512
                w1 = min(NSH, w0 + 512)
                pa = sap.tile([16, 512], f32, tag="pa")
                nc.tensor.matmul(pa[:, : w1 - w0], A1_sb[:], h1T_sb[:, w0:w1],
                                 start=True, stop=True)
                nc.vector.tensor_copy(al_sb[:, w0:w1], pa[:, : w1 - w0])

            for w in range(NWIN):
                w0 = w * WIN
                wn = min(WIN, NSH - w0)
                hp = sap.tile([128, 128], f32, tag="hp")
                if has_bias:
                    nc.tensor.matmul(hp[:wn, :], xT_sb[:, w0:w0 + wn], W1_sb[:],
                                     start=True, stop=False)
                    nc.tensor.matmul(hp[:wn, :], ones_sb[0:1, :wn], b1_sb[:],
                                     start=False, stop=True)
                else:
                    nc.tensor.matmul(hp[:wn, :], xT_sb[:, w0:w0 + wn], W1_sb[:],
                                     start=True, stop=True)
                at = sap.tile([128, 16], f32, tag="at")
                nc.tensor.transpose(at[:wn, :], al_sb[:, w0:w0 + wn], id_sb[:16, :16])
                rowt = rows.tile([128, R1], bf16, tag="rowt")
                nc.vector.tensor_copy(rowt[:wn, 0:16].bitcast(f32), at[:wn, 0:8])
                nc.scalar.copy(rowt[:wn, 16:144], hp[:wn, :])
                nc.vector.memset(rowt[:wn, 144:R1], 0.0)
                nc.sync.dma_start(t1shard.opt()[w0:w0 + wn, :], rowt[:wn, :])
                nc.vector.tensor_copy(als_sb[:wn, w * 16:(w + 1) * 16], at[:wn, :])
                nc.vector.tensor_copy(aldw_bf[:wn, w * 8:(w + 1) * 8],
                                      at[:wn, 8:16])
                nc.vector.tensor_copy(h1w_sb[:wn, w * 128:(w + 1) * 128],
                                      hp[:wn, :])

        from concourse import library_config

        nc.gpsimd.load_library(library_config.mlp)

        nc.gpsimd.collective_compute(
            "AllGather", mybir.AluOpType.bypass,
            replica_groups=[list(range(cfg.NCORES))],
            ins=[t1shard.opt()], outs=[t1full.opt()],
        )

        # ------- edge layers -------
        def edge_layer(table_full, row, rdt, alst, hoff, nh, chper, msgc,
                       alw_bf, epilogue):
            with tc.tile_pool(name="edges", bufs=7) as epool, \
                 tc.tile_pool(name="emsg", bufs=4) as mp, \
                 tc.tile_pool(name="epsum", bufs=meta["max_inflight"] + 1,
                              space="PSUM") as pp, \
                 tc.tile_pool(name="aldpsum", bufs=2, space="PSUM") as pae, \
                 tc.tile_pool(name="esmall", bufs=4) as spool:
                pw_by_w = {}
                qctr = [0]
                for g in range(NG):
                    na = nA[g]
                    gb = epool.tile([128, SC * row], rdt, tag="gb")
                    gb3 = gb[:].rearrange("p (k e) -> p k e", k=SC)
                    c0 = g * SC * 8
                    for grp in range(2):
                        nch = na if grp == 0 else SC - na
                        if nch == 0:
                            continue
                        ksl = slice(0, na) if grp == 0 else slice(na, SC)
                        csl = (slice(c0, c0 + na * 8) if grp == 0
                               else slice(c0 + na * 8, c0 + SC * 8))
                        tbl = (table_full.opt() if grp == 0
                               else table_full.opt()[cfg.SPLIT:N, :])
                        nc.gpsimd.dma_gather(
                            gb3[:, ksl, :], tbl, src_sb[:, csl],
                            num_idxs=nch * 128, num_idxs_reg=nch * 128,
                            elem_size=row, single_packet=False,
                            queue_num=qctr[0] % NQ,
                        )
                        qctr[0] += 1
                    # S0T [j, e] streamed from DRAM (host-baked)
                    s0t = epool.tile([128, SC * 128], fp8, tag="s0t")
                    nc.sync.dma_start(
                        s0t[:], ins["s0t_bf"][:, g * SC * 128:(g + 1) * SC * 128])
                    # al_d per edge via PE: alde[e, h] = sum_j S0T[j,e] ald[j,h]
                    pa = pae.tile([128, SC * nh], f32, tag="pa", name="pa")
                    for k in range(SC):
                        w = cmap[g * SC + k][0]
                        nc.tensor.matmul(
                            pa[:, k * nh:(k + 1) * nh],
                            s0t[:, k * 128:(k + 1) * 128],
                            alw_bf[:, w * nh:(w + 1) * nh],
                            start=True, stop=True)
                    lg = spool.tile([128, SC * nh], f32, tag="lg")
                    nc.vector.tensor_tensor(
                        _ap(lg[:], [[nh, SC], [1, nh]]),
                        _ap(gb[:].bitcast(f32), [[alst, SC], [1, nh]]),
                        _ap(pa[:], [[nh, SC], [1, nh]]),
                        OP.add,
                    )
                    lr = spool.tile([128, SC * nh], f32, tag="lr")
                    nc.vector.scalar_tensor_tensor(
                        lr[:], lg[:], 0.2, lg[:], OP.mult, OP.max)
                    ee = spool.tile([128, SC * nh], bf16, tag="ee")
                    nc.scalar.activation(ee[:], lr[:], AF.Exp)
                    msg = mp.tile([128, SC * msgc], bf16, tag="msg")
                    nc.vector.tensor_tensor(
                        _ap(msg[:], [[msgc, SC], [chper, nh], [1, chper]]),
                        _ap(gb[:], [[row, SC], [chper, nh], [1, chper]], hoff),
                        _ap(ee[:], [[nh, SC], [1, nh], [0, chper]]),
                        OP.mult,
                    )
                    nc.scalar.copy(
                        _ap(msg[:], [[msgc, SC], [1, nh]], msgc - nh),
                        _ap(ee[:], [[nh, SC], [1, nh]]),
                    )
                    s0 = epool.tile([128, SC * 128], bf16, tag="s0")
                    nc.vector.tensor_tensor(
                        _ap(s0[:], [[128, SC], [1, 128]]),
                        _ap(iota_bf[:], [[0, SC], [1, 128]]),
                        _ap(rel_sb[:, g * SC:(g + 1) * SC], [[1, SC], [0, 128]]),
                        OP.is_equal,
                    )
                    for k in range(SC):
                        kk = g * SC + k
                        w, first, last = cmap[kk]
                        if first:
                            pw_by_w[w] = pp.tile([128, msgc], f32, tag="pw", name="pw")
                        pw = pw_by_w[w]
                        nc.tensor.matmul(
                            pw[:], s0[:, k * 128:(k + 1) * 128],
                            msg[:, k * msgc:(k + 1) * msgc],
                            start=first, stop=last,
                        )
                        if last:
                            epilogue(w, pw_by_w.pop(w))

        # ---- L1 ----
        with tc.tile_pool(name="epi1", bufs=2) as hq, \
             tc.tile_pool(name="epi1p", bufs=1, space="PSUM") as hpp:
            def epi1(w, pw):
                w0 = w * WIN
                wn = min(WIN, NSH - w0)
                # self-loop term
                lgs = hq.tile([128, 8], f32, tag="lgs")
                nc.vector.tensor_tensor(lgs[:], als_sb[:, w * 16:w * 16 + 8],
                                        als_sb[:, w * 16 + 8:w * 16 + 16], OP.add)
                ees = hq.tile([128, 8], f32, tag="ees")
                nc.vector.scalar_tensor_tensor(
                    ees[:], lgs[:], 0.2, lgs[:], OP.mult, OP.max)
                nc.scalar.activation(ees[:], ees[:], AF.Exp)
                dn = hq.tile([128, 8], f32, tag="dn")
                nc.vector.scalar_tensor_tensor(
                    dn[:], pw[:, 128:136], 1e-16, ees[:], OP.add, OP.add)
                rcp = hq.tile([128, 8], f32, tag="rcp")
                nc.vector.reciprocal(rcp[:], dn[:])
                uu = hq.tile([128, 128], f32, tag="uu")
                nc.vector.tensor_tensor(
                    _ap(uu[:], [[16, 8], [1, 16]]),
                    _ap(h1w_sb[:, w * 128:(w + 1) * 128], [[16, 8], [1, 16]]),
                    _ap(ees[:], [[1, 8], [0, 16]]),
                    OP.mult,
                )
                nc.vector.tensor_tensor(uu[:], uu[:], pw[:, 0:128], OP.add)
                hb = hq.tile([128, 128], f32, tag="hb")
                nc.vector.tensor_tensor(
                    _ap(hb[:], [[16, 8], [1, 16]]),
                    _ap(uu[:], [[16, 8], [1, 16]]),
                    _ap(rcp[:], [[1, 8], [0, 16]]),
                    OP.mult,
                )
                nc.scalar.activation(hb[:], hb[:], AF.Relu)
                tp = hpp.tile([128, 128], f32, tag="tp")
                nc.tensor.transpose(tp[:], hb[:], id_sb[:])
                tH = hq.tile([128, 128], f32, tag="tH")
                nc.vector.tensor_copy(tH[:], tp[:])
                p2 = hpp.tile([128, 66], f32, tag="p2")
                if has_bias:
                    nc.tensor.matmul(p2[:], tH[:], RHS2_sb[:], start=True, stop=False)
                    nc.tensor.matmul(p2[:], ones_sb[0:1, :128], b2_sb[:],
                                     start=False, stop=True)
                else:
                    nc.tensor.matmul(p2[:], tH[:], RHS2_sb[:], start=True, stop=True)
                t2b = hq.tile([128, R2], bf16, tag="t2b")
                nc.vector.tensor_copy(t2b[:wn, 0:2].bitcast(f32), p2[:wn, 0:1])
                nc.scalar.copy(t2b[:wn, 2:66], p2[:wn, 2:66])
                nc.vector.memset(t2b[:wn, 66:R2], 0.0)
                nc.sync.dma_start(t2shard.opt()[w0:w0 + wn, :], t2b[:wn, :])
                nc.vector.tensor_copy(al2_sb[:wn, w * 2:(w + 1) * 2], p2[:wn, 0:2])
                nc.vector.tensor_copy(al2_bf[:wn, w:w + 1], p2[:wn, 1:2])
                nc.vector.tensor_copy(h2w_sb[:wn, w * 64:(w + 1) * 64],
                                      p2[:wn, 2:66])

            edge_layer(t1full, cfg.R1, mybir.dt.bfloat16, 128, cfg.H1OFF,
                       8, 16, M1, aldw_bf, epi1)

        nc.gpsimd.collective_compute(
            "AllGather", mybir.AluOpType.bypass,
            replica_groups=[list(range(cfg.NCORES))],
            ins=[t2shard.opt()], outs=[t2full.opt()],
        )

        # ---- L2 ----
        with tc.tile_pool(name="epi2", bufs=2) as oq:
            def epi2(w, pw):
                w0 = w * WIN
                wn = min(WIN, NSH - w0)
                lg2 = oq.tile([128, 1], f32, tag="lg2")
                nc.vector.tensor_tensor(lg2[:], al2_sb[:, w * 2:w * 2 + 1],
                                        al2_sb[:, w * 2 + 1:w * 2 + 2], OP.add)
                ee2 = oq.tile([128, 1], f32, tag="ee2")
                nc.vector.scalar_tensor_tensor(
                    ee2[:], lg2[:], 0.2, lg2[:], OP.mult, OP.max)
                nc.scalar.activation(ee2[:], ee2[:], AF.Exp)
                dn2 = oq.tile([128, 1], f32, tag="dn2")
                nc.vector.scalar_tensor_tensor(
                    dn2[:], pw[:, 64:65], 1e-16, ee2[:], OP.add, OP.add)
                rcp2 = oq.tile([128, 1], f32, tag="rcp2")
                nc.vector.reciprocal(rcp2[:], dn2[:])
                ms2 = oq.tile([128, 64], f32, tag="ms2")
                nc.vector.tensor_tensor(
                    ms2[:], h2w_sb[:, w * 64:(w + 1) * 64],
                    _ap(ee2[:], [[0, 64]]), OP.mult)
                nc.vector.tensor_tensor(ms2[:], ms2[:], pw[:, 0:64], OP.add)
                ob = oq.tile([128, 64], f32, tag="ob")
                nc.vector.tensor_tensor(ob[:], ms2[:], _ap(rcp2[:], [[0, 64]]),
                                        OP.mult)
                nc.sync.dma_start(out_ap[w0:w0 + wn, :], ob[:wn, :])

            edge_layer(t2full, cfg.R2, mybir.dt.bfloat16, 64, cfg.H2OFF,
                       1, 64, M2, al2_bf, epi2)


# ---------------------------------------------------------------------------
# SPMD build + run
# ---------------------------------------------------------------------------

_CACHE = {}


def _build(meta, cfg: Cfg, has_bias: bool):
    key = (tuple(meta["cmap"]), tuple(meta["nA"]), cfg.N, cfg.NCORES, has_bias)
    if key in _CACHE:
        return _CACHE[key]
    import concourse.tile as tile
    from concourse import bacc, mybir

    f32 = mybir.dt.float32
    i16 = mybir.dt.int16
    TC = meta["TC"]
    NIW = TC * cfg.CHUNK // 16
    nc = bacc.Bacc("TRN2", target_bir_lowering=False, debug=False,
                   num_devices=cfg.NCORES,
                   num_swdge_queues=getattr(cfg, "NQUEUES", 1))
    ins = {}

    def di(name, shape, dt=f32):
        ins[name] = nc.dram_tensor(name, shape, dt, kind="ExternalInput").ap()

    di("xT", [128, cfg.NSH])
    di("W1", [128, 128])
    di("A1", [128, 16])
    di("RHS2", [128, 66])
    di("src_idx", [128, NIW], i16)
    di("rel_bf", [128, TC])
    di("s0t_bf", [128, TC * 128], mybir.dt.float8e4)
    if has_bias:
        di("b1row", [1, 128])
        di("b2row", [1, 66])
    out = nc.dram_tensor("out", [cfg.NSH, cfg.OUT_C], f32, kind="ExternalOutput").ap()

    with tile.TileContext(nc) as tc:
        emit_gat(tc, out, ins, meta, cfg, has_bias)
    nc.compile()
    _CACHE[key] = nc
    return nc


def kernel(**inputs) -> np.ndarray:
    out, _ = _run(inputs)
    return out


def _run(inputs, **run_kwargs):
    cfg = FULL
    inputs = {k: np.asarray(v) for k, v in inputs.items()}
    edge_index = inputs["edge_index"].astype(np.int64)
    meta, per_core = prep_edges(edge_index, cfg)
    shared, xT, has_bias = host_tensors(inputs, cfg)
    nc = _build(meta, cfg, has_bias)

    from concourse.bass_utils import run_bass_kernel_spmd

    import ml_dtypes

    in_maps = []
    for c in range(cfg.NCORES):
        m = {k: shared[k] for k in ("W1", "A1", "RHS2")}
        if has_bias:
            m["b1row"] = shared["b1row"]
            m["b2row"] = shared["b2row"]
        m["xT"] = xT[c]
        pc = dict(per_core[c])
        pc["s0t_bf"] = pc["s0t_bf"].view(ml_dtypes.float8_e4m3fn)
        m.update(pc)
        in_maps.append(m)
    res = run_bass_kernel_spmd(nc, in_maps, core_ids=list(range(cfg.NCORES)),
                               **run_kwargs)
    out = np.concatenate([res.results[c]["out"] for c in range(cfg.NCORES)], 0)
    return out.astype(np.float32), res


# revision 17
# speedup vs baseline: 1.0386x; 1.0326x over previous
"""Trainium2 Bass kernel for 2-layer GAT (nn_GAT_72619307041134).

Strategy (dst-sharded edge parallelism, 8 cores SPMD):
- Nodes sharded into 8 contiguous ranges of 6250; edges sorted by dst and
  sharded by dst range, so each core owns ALL edges of its dst nodes and the
  segment softmax + aggregation need no cross-core reduction.
- Self-loops are NOT in the edge stream; their contribution is folded into
  the window epilogues (per-node dense math), saving ~6% of gather traffic.
- Per layer, a per-node gather table lives in DRAM:
    table1[n] = [al_src (8 bf16) | h1 (128 fp8)] 256B fp8 rows
    table2[n] = [al2_src (1 f32 = 2 slots) | h2 (64 bf16) | pad] 256B rows
  The dst-side attention logits (al_d) are NOT gathered per edge: within a
  128-node dst window they are broadcast to edges with a small PE matmul
  alde[e,h] = sum_j S0T[j,e] * al_d[j,h], where S0T is the transposed
  edge->node indicator, host-baked as exact fp8 0/1 and streamed from DRAM
  on the (otherwise idle) HWDGE path.
- Edges are processed in 128-edge chunks (16 chunks = 1 super-chunk):
  dma_gather fetches the src rows (int16 indices; srcs >= SPLIT gather
  from a shifted view; A/B-pure chunks, A-first in each super-chunk),
  each call round-robined over all 4 SWDGE queues (the drain of these
  random 256-512B HBM reads is the kernel's bottleneck).
  DVE computes ee = exp(leaky_relu(al_s+al_d)) and msg = [ee*h | ee] (bf16);
  an indicator S0[e,j] = (dst_rel[e]==j) turns the per-window segment-sum
  into PE matmuls accumulating in PSUM.  Window epilogue adds the self-loop
  term and divides by the softmax denominator (segment-max shift skipped;
  logits are bounded by ~4).
- Pad edges have dst_rel=-1 (S0/S0T column zero) and gather row 0.
- Between layers one AllGather shares each core's table2 shard.
- Host preprocessing only sorts/pads/permutes integer indices.
"""

from contextlib import ExitStack

import numpy as np

# ---------------------------------------------------------------------------
# config
# ---------------------------------------------------------------------------


class Cfg:
    def __init__(self, N=50000, E=800000, NCORES=8, WIN=128, CHUNK=128, SC=16,
                 SPLIT=32768):
        self.N = N
        self.E = E
        self.NCORES = NCORES
        self.NSH = N // NCORES
        self.WIN = WIN
        self.NWIN = (self.NSH + WIN - 1) // WIN
        self.CHUNK = CHUNK
        self.SC = SC
        self.SPLIT = SPLIT        # src < SPLIT -> table A view, else B view
        self.HEADS = 8
        self.HID = 16
        self.OUT_C = 64
        self.R1 = 256             # bf16 elems/row: [al_s 8 f32 = 16 slots | h1 128 bf16 | pad]
        self.H1OFF = 16
        self.R2 = 128             # bf16 elems/row: [al2_s 2 slots | h2 64 | pad]
        self.H2OFF = 2
        self.MSG1 = 128 + 8
        self.MSG2 = 64 + 1
        self.NQUEUES = 4


FULL = Cfg()

# ---------------------------------------------------------------------------
# host-side edge preprocessing (indices only)
# ---------------------------------------------------------------------------


def prep_edges(edge_index: np.ndarray, cfg: Cfg):
    """Builds the SPMD-common chunk schedule and per-core index arrays.

    meta: cmap (slot -> (window, first, last)), nA (A-chunks per super-chunk,
    A-first slot order), TC, NG.
    per_core[c]: int16 'src_idx' [16, EPC/16] (wrapped: edge i of the
    slot-ordered stream at [i%16, i//16]), f32 'dst_rel' [128, TC]
    (rel[e, slot] for the edge at (slot, partition e), -1 for pads),
    int16 's0t_bits' [128, TC*8] (bit (j, slot, e) = (rel[slot,e]==j),
    packed 16 edges/word along e).
    """
    N, NSH, WIN, CHUNK, SC = cfg.N, cfg.NSH, cfg.WIN, cfg.CHUNK, cfg.SC
    src = edge_index[0].astype(np.int64)
    dst = edge_index[1].astype(np.int64)
    order = np.argsort(dst, kind="stable")
    src = src[order]
    dst = dst[order]
    isA = src < cfg.SPLIT
    core_of = dst // NSH
    w_of = (dst % NSH) // WIN
    cntA = np.zeros((cfg.NCORES, cfg.NWIN), np.int64)
    cntB = np.zeros((cfg.NCORES, cfg.NWIN), np.int64)
    np.add.at(cntA, (core_of[isA], w_of[isA]), 1)
    np.add.at(cntB, (core_of[~isA], w_of[~isA]), 1)
    cpwA = np.ceil(cntA.max(0) / CHUNK).astype(np.int64)
    cpwB = np.ceil(cntB.max(0) / CHUNK).astype(np.int64)
    tc = int((cpwA + cpwB).sum())
    cpwA[-1] += (-tc) % SC
    TC = int((cpwA + cpwB).sum())
    NG = TC // SC

    # global chunk list: window-major, A then B inside a window
    glist = []  # (window, is_A)
    for w in range(cfg.NWIN):
        glist += [(w, True)] * int(cpwA[w]) + [(w, False)] * int(cpwB[w])
    # per-super-chunk stable reorder: A-chunks first
    slot_of = []
    nA = []
    for g in range(NG):
        blk = list(range(g * SC, (g + 1) * SC))
        a = [i for i in blk if glist[i][1]]
        b = [i for i in blk if not glist[i][1]]
        nA.append(len(a))
        slot_of += a + b
    lastslot = {}
    for s, gi in enumerate(slot_of):
        lastslot[glist[gi][0]] = s
    cmap = []
    seen = set()
    inflight = 0
    max_inflight = 0
    for s, gi in enumerate(slot_of):
        w = glist[gi][0]
        first = w not in seen
        seen.add(w)
        last = lastslot[w] == s
        if first:
            inflight += 1
            max_inflight = max(max_inflight, inflight)
        cmap.append((w, first, last))
        if last:
            inflight -= 1

    starts = np.searchsorted(
        dst, (np.arange(0, N, NSH)[:, None] + np.arange(0, NSH, WIN)[None, :]))
    flat_starts = list(starts.ravel()) + [len(dst)]
    per_core = []
    for c in range(cfg.NCORES):
        s_by_g = np.zeros((len(glist), CHUNK), np.int64)
        r_by_g = np.full((len(glist), CHUNK), -1, np.int64)
        gi = 0
        for w in range(cfg.NWIN):
            i = c * cfg.NWIN + w
            s0, s1 = flat_starts[i], flat_starts[i + 1]
            sl = src[s0:s1]
            dl = dst[s0:s1]
            il = isA[s0:s1]
            for grp in (True, False):
                m = il == grp
                se = sl[m]
                de = dl[m]
                nch = int(cpwA[w]) if grp else int(cpwB[w])
                buf_s = np.zeros(nch * CHUNK, np.int64)
                buf_r = np.full(nch * CHUNK, -1, np.int64)
                buf_s[:len(se)] = se - (0 if grp else cfg.SPLIT)
                buf_r[:len(se)] = de - (c * NSH + w * WIN)
                s_by_g[gi:gi + nch] = buf_s.reshape(nch, CHUNK)
                r_by_g[gi:gi + nch] = buf_r.reshape(nch, CHUNK)
                gi += nch
        svals = s_by_g[slot_of]   # [TC, 128] slot-ordered
        rvals = r_by_g[slot_of]

        def wrap(vals):
            st = vals.reshape(-1)   # stream i = slot*128 + p
            n = st.shape[0]
            out = np.zeros((16, n // 16), np.int16)
            idx = np.arange(n)
            out[idx % 16, idx // 16] = st
            return np.tile(out, (8, 1))

        # host-baked transposed one-hot (fp8 e4m3 bit pattern: 1.0 = 0x38)
        ONE = np.uint8(0x38)
        jj = np.arange(128, dtype=np.int64)
        # s0t[j, slot*128+e] = (rvals[slot, e] == j)
        s0t = (rvals[None, :, :] == jj[:, None, None]).astype(np.uint8) * ONE
        per_core.append(dict(
            src_idx=wrap(svals),
            rel_bf=np.ascontiguousarray(rvals.T).astype(np.float32),
            s0t_bf=np.ascontiguousarray(s0t.reshape(128, TC * 128)),
        ))
    meta = dict(cmap=cmap, nA=nA, TC=TC, NG=NG, max_inflight=max_inflight)
    return meta, per_core


def host_tensors(inputs, cfg: Cfg):
    x = np.ascontiguousarray(inputs["x"], np.float32)
    W1 = np.ascontiguousarray(inputs["W1"], np.float32)
    a1s = np.asarray(inputs["a1_src"], np.float32)
    a1d = np.asarray(inputs["a1_dst"], np.float32)
    W2 = np.ascontiguousarray(inputs["W2"], np.float32)
    a2s = np.asarray(inputs["a2_src"], np.float32).reshape(1, -1)
    a2d = np.asarray(inputs["a2_dst"], np.float32).reshape(1, -1)
    b1 = np.asarray(inputs["b1"], np.float32)
    b2 = np.asarray(inputs["b2"], np.float32)
    H, HID = cfg.HEADS, cfg.HID
    A1 = np.zeros((H * HID, 2 * H), np.float32)
    for h in range(H):
        A1[h * HID:(h + 1) * HID, h] = a1s[h]
        A1[h * HID:(h + 1) * HID, H + h] = a1d[h]
    RHS2 = np.concatenate([W2 @ a2s.T, W2 @ a2d.T, W2], 1).astype(np.float32)
    shared = dict(W1=W1, A1=A1, RHS2=RHS2,
                  b1row=b1.reshape(1, -1),
                  b2row=np.concatenate([np.zeros(2, np.float32), b2]).reshape(1, -1))
    xT = [np.ascontiguousarray(x[c * cfg.NSH:(c + 1) * cfg.NSH].T)
          for c in range(cfg.NCORES)]
    has_bias = bool(np.any(b1) or np.any(b2))
    return shared, xT, has_bias


# ---------------------------------------------------------------------------
# device kernel emission
# ---------------------------------------------------------------------------


def _ap(base, free_dims, extra_off=0):
    """Replace the free dims of a [P, ...] AP (keep partition dim)."""
    import concourse.bass as bass

    return bass.AP(base.tensor, base.offset + extra_off,
                   [list(base.ap[0])] + [list(d) for d in free_dims])


def emit_gat(tc, out_ap, ins, meta, cfg: Cfg, has_bias=False):
    import concourse.bass as bass  # noqa: F401
    from concourse import mybir

    nc = tc.nc
    f32 = mybir.dt.float32
    bf16 = mybir.dt.bfloat16
    fp8 = mybir.dt.float8e4
    i16 = mybir.dt.int16
    i32 = mybir.dt.int32
    AF = mybir.ActivationFunctionType
    OP = mybir.AluOpType
    N, NSH, WIN, NWIN, SC = cfg.N, cfg.NSH, cfg.WIN, cfg.NWIN, cfg.SC
    TC, NG = meta["TC"], meta["NG"]
    cmap, nA = meta["cmap"], meta["nA"]
    NQ = getattr(cfg, "NQUEUES", 1)
    R1, R2 = cfg.R1, cfg.R2
    M1, M2 = cfg.MSG1, cfg.MSG2
    NIW = TC * cfg.CHUNK // 16

    ctx = ExitStack()
    with ctx:
        dram = ctx.enter_context(tc.tile_pool(name="dram", bufs=1, space="DRAM"))
        consts = ctx.enter_context(tc.tile_pool(name="consts", bufs=1))

        t1shard = dram.tile([NSH, R1], bf16)
        t1full = dram.tile([N, R1], bf16, addr_space="Shared")
        t2shard = dram.tile([NSH, R2], bf16)
        t2full = dram.tile([N, R2], bf16, addr_space="Shared")

        # ------- constants into SBUF -------
        W1_sb = consts.tile([128, 128], f32)
        A1_sb = consts.tile([128, 16], f32)
        RHS2_sb = consts.tile([128, 66], f32)
        nc.sync.dma_start(W1_sb[:], ins["W1"][:])
        nc.sync.dma_start(A1_sb[:], ins["A1"][:])
        nc.sync.dma_start(RHS2_sb[:], ins["RHS2"][:])
        src_sb = consts.tile([128, NIW], i16)
        nc.sync.dma_start(src_sb[:], ins["src_idx"][:])
        relf_sb = consts.tile([128, TC], f32)
        rel_sb = consts.tile([128, TC], bf16)
        nc.sync.dma_start(relf_sb[:], ins["rel_bf"][:])
        if has_bias:
            b1_sb = consts.tile([1, 128], f32)
            b2_sb = consts.tile([1, 66], f32)
            ones_sb = consts.tile([1, 128], f32)
            nc.sync.dma_start(b1_sb[:], ins["b1row"][:])
            nc.sync.dma_start(b2_sb[:], ins["b2row"][:])
            nc.gpsimd.memset(ones_sb[:], 1.0)

        # identity + iota + bitmask
        iota_i = consts.tile([128, 128], i32)
        icol = consts.tile([128, 1], i32)
        id_sb = consts.tile([128, 128], f32)
        iota_f = consts.tile([128, 128], f32)
        icol_f = consts.tile([128, 1], f32)
        nc.gpsimd.iota(iota_i[:], pattern=[[1, 128]], base=0, channel_multiplier=0)
        nc.gpsimd.iota(icol[:], pattern=[[1, 1]], base=0, channel_multiplier=1)
        nc.vector.tensor_copy(iota_f[:], iota_i[:])
        nc.vector.tensor_copy(icol_f[:], icol[:])
        nc.vector.tensor_scalar(id_sb[:], iota_f[:], icol_f[:], None, OP.is_equal)
        iota_bf = consts.tile([128, 128], bf16)
        nc.vector.tensor_copy(iota_bf[:], iota_f[:])
        nc.vector.tensor_copy(rel_sb[:], relf_sb[:])

        # persistent per-node state (per window layout)
        als_sb = consts.tile([128, NWIN * 16], f32)     # [al_s 8 | al_d 8]
        aldw_bf = consts.tile([128, NWIN * 8], bf16)    # al_d (L1)
        al2_sb = consts.tile([128, NWIN * 2], f32)      # [al2_s, al2_d]
        al2_bf = consts.tile([128, NWIN], bf16)         # al2_d
        h1w_sb = consts.tile([128, NWIN * 128], bf16)   # h1 rows per window
        h2w_sb = consts.tile([128, NWIN * 64], bf16)    # h2 rows per window
        nc.vector.memset(aldw_bf[:], 0.0)
        nc.vector.memset(al2_bf[:], 0.0)

        # ------- stage A: h1, al, table1 -------
        with tc.tile_pool(name="stageA", bufs=1) as sa, \
             tc.tile_pool(name="stageApsum", bufs=2, space="PSUM") as sap, \
             tc.tile_pool(name="rows", bufs=3) as rows:
            xT_sb = sa.tile([128, NSH], f32)
            nc.sync.dma_start(xT_sb[:], ins["xT"][:])
            h1T_sb = sa.tile([128, NSH], f32)
            al_sb = sa.tile([16, NSH], f32)
            nt = (NSH + 511) // 512
            for j in range(nt):
                w0 = j * 512
                w1 = min(NSH, w0 + 512)
                ph = sap.tile([128, 512], f32, tag="ph")
                nc.tensor.matmul(ph[:, : w1 - w0], W1_sb[:], xT_sb[:, w0:w1],
                                 start=True, stop=True)
                nc.vector.tensor_copy(h1T_sb[:, w0:w1], ph[:, : w1 - w0])
            for j in range(nt):
                w0 = j * 512
                w1 = min(NSH, w0 + 512)
                pa = sap.tile([16, 512], f32, tag="pa")
                nc.tensor.matmul(pa[:, : w1 - w0], A1_sb[:], h1T_sb[:, w0:w1],
                                 start=True, stop=True)
                nc.vector.tensor_copy(al_sb[:, w0:w1], pa[:, : w1 - w0])

            for w in range(NWIN):
                w0 = w * WIN
                wn = min(WIN, NSH - w0)
                hp = sap.tile([128, 128], f32, tag="hp")
                if has_bias:
                    nc.tensor.matmul(hp[:wn, :], xT_sb[:, w0:w0 + wn], W1_sb[:],
                                     start=True, stop=False)
                    nc.tensor.matmul(hp[:wn, :], ones_sb[0:1, :wn], b1_sb[:],
                                     start=False, stop=True)
                else:
                    nc.tensor.matmul(hp[:wn, :], xT_sb[:, w0:w0 + wn], W1_sb[:],
                                     start=True, stop=True)
                at = sap.tile([128, 16], f32, tag="at")
                nc.tensor.transpose(at[:wn, :], al_sb[:, w0:w0 + wn], id_sb[:16, :16])
                rowt = rows.tile([128, R1], bf16, tag="rowt")
                nc.vector.tensor_copy(rowt[:wn, 0:16].bitcast(f32), at[:wn, 0:8])
                nc.scalar.copy(rowt[:wn, 16:144], hp[:wn, :])
                nc.vector.memset(rowt[:wn, 144:R1], 0.0)
                nc.sync.dma_start(t1shard.opt()[w0:w0 + wn, :], rowt[:wn, :])
                nc.vector.tensor_copy(als_sb[:wn, w * 16:(w + 1) * 16], at[:wn, :])
                nc.vector.tensor_copy(aldw_bf[:wn, w * 8:(w + 1) * 8],
                                      at[:wn, 8:16])
                nc.vector.tensor_copy(h1w_sb[:wn, w * 128:(w + 1) * 128],
                                      hp[:wn, :])

        from concourse import library_config

        nc.gpsimd.load_library(library_config.mlp)

        nc.gpsimd.collective_compute(
            "AllGather", mybir.AluOpType.bypass,
            replica_groups=[list(range(cfg.NCORES))],
            ins=[t1shard.opt()], outs=[t1full.opt()],
        )

        # ------- edge layers -------
        def edge_layer(table_full, row, rdt, alst, hoff, nh, chper, msgc,
                       alw_bf, epilogue):
            with tc.tile_pool(name="edges", bufs=7) as epool, \
                 tc.tile_pool(name="emsg", bufs=4) as mp, \
                 tc.tile_pool(name="epsum", bufs=meta["max_inflight"] + 1,
                              space="PSUM") as pp, \
                 tc.tile_pool(name="aldpsum", bufs=2, space="PSUM") as pae, \
                 tc.tile_pool(name="esmall", bufs=4) as spool:
                pw_by_w = {}
                qctr = [0]
                for g in range(NG):
                    na = nA[g]
                    gb = epool.tile([128, SC * row], rdt, tag="gb")
                    gb3 = gb[:].rearrange("p (k e) -> p k e", k=SC)
                    c0 = g * SC * 8
                    for grp in range(2):
                        nch = na if grp == 0 else SC - na
                        if nch == 0:
                            continue
                        k0 = 0 if grp == 0 else na
                        tbl = (table_full.opt() if grp == 0
                               else table_full.opt()[cfg.SPLIT:N, :])
                        nh1 = (nch + 1) // 2
                        for h0, hn in ((0, nh1), (nh1, nch - nh1)):
                            if hn == 0:
                                continue
                            ksl = slice(k0 + h0, k0 + h0 + hn)
                            csl = slice(c0 + (k0 + h0) * 8,
                                        c0 + (k0 + h0 + hn) * 8)
                            nc.gpsimd.dma_gather(
                                gb3[:, ksl, :], tbl, src_sb[:, csl],
                                num_idxs=hn * 128, num_idxs_reg=hn * 128,
                                elem_size=row, single_packet=False,
                                queue_num=qctr[0] % NQ,
                            )
                            qctr[0] += 1
                    # S0T [j, e] streamed from DRAM (host-baked)
                    s0t = epool.tile([128, SC * 128], fp8, tag="s0t")
                    nc.sync.dma_start(
                        s0t[:], ins["s0t_bf"][:, g * SC * 128:(g + 1) * SC * 128])
                    # al_d per edge via PE: alde[e, h] = sum_j S0T[j,e] ald[j,h]
                    pa = pae.tile([128, SC * nh], f32, tag="pa", name="pa")
                    for k in range(SC):
                        w = cmap[g * SC + k][0]
                        nc.tensor.matmul(
                            pa[:, k * nh:(k + 1) * nh],
                            s0t[:, k * 128:(k + 1) * 128],
                            alw_bf[:, w * nh:(w + 1) * nh],
                            start=True, stop=True)
                    lg = spool.tile([128, SC * nh], f32, tag="lg")
                    nc.vector.tensor_tensor(
                        _ap(lg[:], [[nh, SC], [1, nh]]),
                        _ap(gb[:].bitcast(f32), [[alst, SC], [1, nh]]),
                        _ap(pa[:], [[nh, SC], [1, nh]]),
                        OP.add,
                    )
                    lr = spool.tile([128, SC * nh], f32, tag="lr")
                    nc.vector.scalar_tensor_tensor(
                        lr[:], lg[:], 0.2, lg[:], OP.mult, OP.max)
                    ee = spool.tile([128, SC * nh], bf16, tag="ee")
                    nc.scalar.activation(ee[:], lr[:], AF.Exp)
                    msg = mp.tile([128, SC * msgc], bf16, tag="msg")
                    nc.vector.tensor_tensor(
                        _ap(msg[:], [[msgc, SC], [chper, nh], [1, chper]]),
                        _ap(gb[:], [[row, SC], [chper, nh], [1, chper]], hoff),
                        _ap(ee[:], [[nh, SC], [1, nh], [0, chper]]),
                        OP.mult,
                    )
                    nc.scalar.copy(
                        _ap(msg[:], [[msgc, SC], [1, nh]], msgc - nh),
                        _ap(ee[:], [[nh, SC], [1, nh]]),
                    )
                    s0 = epool.tile([128, SC * 128], bf16, tag="s0")
                    nc.vector.tensor_tensor(
                        _ap(s0[:], [[128, SC], [1, 128]]),
                        _ap(iota_bf[:], [[0, SC], [1, 128]]),
                        _ap(rel_sb[:, g * SC:(g + 1) * SC], [[1, SC], [0, 128]]),
                        OP.is_equal,
                    )
                    for k in range(SC):
                        kk = g * SC + k
                        w, first, last = cmap[kk]
                        if first:
                            pw_by_w[w] = pp.tile([128, msgc], f32, tag="pw", name="pw")
                        pw = pw_by_w[w]
                        nc.tensor.matmul(
                            pw[:], s0[:, k * 128:(k + 1) * 128],
                            msg[:, k * msgc:(k + 1) * msgc],
                            start=first, stop=last,
                        )
                        if last:
                            epilogue(w, pw_by_w.pop(w))

        # ---- L1 ----
        with tc.tile_pool(name="epi1", bufs=2) as hq, \
             tc.tile_pool(name="epi1p", bufs=1, space="PSUM") as hpp:
            def epi1(w, pw):
                w0 = w * WIN
                wn = min(WIN, NSH - w0)
                # self-loop term
                lgs = hq.tile([128, 8], f32, tag="lgs")
                nc.vector.tensor_tensor(lgs[:], als_sb[:, w * 16:w * 16 + 8],
                                        als_sb[:, w * 16 + 8:w * 16 + 16], OP.add)
                ees = hq.tile([128, 8], f32, tag="ees")
                nc.vector.scalar_tensor_tensor(
                    ees[:], lgs[:], 0.2, lgs[:], OP.mult, OP.max)
                nc.scalar.activation(ees[:], ees[:], AF.Exp)
                dn = hq.tile([128, 8], f32, tag="dn")
                nc.vector.scalar_tensor_tensor(
                    dn[:], pw[:, 128:136], 1e-16, ees[:], OP.add, OP.add)
                rcp = hq.tile([128, 8], f32, tag="rcp")
                nc.vector.reciprocal(rcp[:], dn[:])
                uu = hq.tile([128, 128], f32, tag="uu")
                nc.vector.tensor_tensor(
                    _ap(uu[:], [[16, 8], [1, 16]]),
                    _ap(h1w_sb[:, w * 128:(w + 1) * 128], [[16, 8], [1, 16]]),
                    _ap(ees[:], [[1, 8], [0, 16]]),
                    OP.mult,
                )
                nc.vector.tensor_tensor(uu[:], uu[:], pw[:, 0:128], OP.add)
                hb = hq.tile([128, 128], f32, tag="hb")
                nc.vector.tensor_tensor(
                    _ap(hb[:], [[16, 8], [1, 16]]),
                    _ap(uu[:], [[16, 8], [1, 16]]),
                    _ap(rcp[:], [[1, 8], [0, 16]]),
                    OP.mult,
                )
                nc.scalar.activation(hb[:], hb[:], AF.Relu)
                tp = hpp.tile([128, 128], f32, tag="tp")
                nc.tensor.transpose(tp[:], hb[:], id_sb[:])
                tH = hq.tile([128, 128], f32, tag="tH")
                nc.vector.tensor_copy(tH[:], tp[:])
                p2 = hpp.tile([128, 66], f32, tag="p2")
                if has_bias:
                    nc.tensor.matmul(p2[:], tH[:], RHS2_sb[:], start=True, stop=False)
                    nc.tensor.matmul(p2[:], ones_sb[0:1, :128], b2_sb[:],
                                     start=False, stop=True)
                else:
                    nc.tensor.matmul(p2[:], tH[:], RHS2_sb[:], start=True, stop=True)
                t2b = hq.tile([128, R2], bf16, tag="t2b")
                nc.vector.tensor_copy(t2b[:wn, 0:2].bitcast(f32), p2[:wn, 0:1])
                nc.scalar.copy(t2b[:wn, 2:66], p2[:wn, 2:66])
                nc.vector.memset(t2b[:wn, 66:R2], 0.0)
                nc.sync.dma_start(t2shard.opt()[w0:w0 + wn, :], t2b[:wn, :])
                nc.vector.tensor_copy(al2_sb[:wn, w * 2:(w + 1) * 2], p2[:wn, 0:2])
                nc.vector.tensor_copy(al2_bf[:wn, w:w + 1], p2[:wn, 1:2])
                nc.vector.tensor_copy(h2w_sb[:wn, w * 64:(w + 1) * 64],
                                      p2[:wn, 2:66])

            edge_layer(t1full, cfg.R1, mybir.dt.bfloat16, 128, cfg.H1OFF,
                       8, 16, M1, aldw_bf, epi1)

        nc.gpsimd.collective_compute(
            "AllGather", mybir.AluOpType.bypass,
            replica_groups=[list(range(cfg.NCORES))],
            ins=[t2shard.opt()], outs=[t2full.opt()],
        )

        # ---- L2 ----
        with tc.tile_pool(name="epi2", bufs=2) as oq:
            def epi2(w, pw):
                w0 = w * WIN
                wn = min(WIN, NSH - w0)
                lg2 = oq.tile([128, 1], f32, tag="lg2")
                nc.vector.tensor_tensor(lg2[:], al2_sb[:, w * 2:w * 2 + 1],
                                        al2_sb[:, w * 2 + 1:w * 2 + 2], OP.add)
                ee2 = oq.tile([128, 1], f32, tag="ee2")
                nc.vector.scalar_tensor_tensor(
                    ee2[:], lg2[:], 0.2, lg2[:], OP.mult, OP.max)
                nc.scalar.activation(ee2[:], ee2[:], AF.Exp)
                dn2 = oq.tile([128, 1], f32, tag="dn2")
                nc.vector.scalar_tensor_tensor(
                    dn2[:], pw[:, 64:65], 1e-16, ee2[:], OP.add, OP.add)
                rcp2 = oq.tile([128, 1], f32, tag="rcp2")
                nc.vector.reciprocal(rcp2[:], dn2[:])
                ms2 = oq.tile([128, 64], f32, tag="ms2")
                nc.vector.tensor_tensor(
                    ms2[:], h2w_sb[:, w * 64:(w + 1) * 64],
                    _ap(ee2[:], [[0, 64]]), OP.mult)
                nc.vector.tensor_tensor(ms2[:], ms2[:], pw[:, 0:64], OP.add)
                ob = oq.tile([128, 64], f32, tag="ob")
                nc.vector.tensor_tensor(ob[:], ms2[:], _ap(rcp2[:], [[0, 64]]),
                                        OP.mult)
                nc.sync.dma_start(out_ap[w0:w0 + wn, :], ob[:wn, :])

            edge_layer(t2full, cfg.R2, mybir.dt.bfloat16, 64, cfg.H2OFF,
                       1, 64, M2, al2_bf, epi2)


# ---------------------------------------------------------------------------
# SPMD build + run
# ---------------------------------------------------------------------------

_CACHE = {}


def _build(meta, cfg: Cfg, has_bias: bool):
    key = (tuple(meta["cmap"]), tuple(meta["nA"]), cfg.N, cfg.NCORES, has_bias)
    if key in _CACHE:
        return _CACHE[key]
    import concourse.tile as tile
    from concourse import bacc, mybir

    f32 = mybir.dt.float32
    i16 = mybir.dt.int16
    TC = meta["TC"]
    NIW = TC * cfg.CHUNK // 16
    nc = bacc.Bacc("TRN2", target_bir_lowering=False, debug=False,
                   num_devices=cfg.NCORES,
                   num_swdge_queues=getattr(cfg, "NQUEUES", 1))
    ins = {}

    def di(name, shape, dt=f32):
        ins[name] = nc.dram_tensor(name, shape, dt, kind="ExternalInput").ap()

    di("xT", [128, cfg.NSH])
    di("W1", [128, 128])
    di("A1", [128, 16])
    di("RHS2", [128, 66])
    di("src_idx", [128, NIW], i16)
    di("rel_bf", [128, TC])
    di("s0t_bf", [128, TC * 128], mybir.dt.float8e4)
    if has_bias:
        di("b1row", [1, 128])
        di("b2row", [1, 66])
    out = nc.dram_tensor("out", [cfg.NSH, cfg.OUT_C], f32, kind="ExternalOutput").ap()

    with tile.TileContext(nc) as tc:
        emit_gat(tc, out, ins, meta, cfg, has_bias)
    nc.compile()
    _CACHE[key] = nc
    return nc


def kernel(**inputs) -> np.ndarray:
    out, _ = _run(inputs)
    return out


def _run(inputs, **run_kwargs):
    cfg = FULL
    inputs = {k: np.asarray(v) for k, v in inputs.items()}
    edge_index = inputs["edge_index"].astype(np.int64)
    meta, per_core = prep_edges(edge_index, cfg)
    shared, xT, has_bias = host_tensors(inputs, cfg)
    nc = _build(meta, cfg, has_bias)

    from concourse.bass_utils import run_bass_kernel_spmd

    import ml_dtypes

    in_maps = []
    for c in range(cfg.NCORES):
        m = {k: shared[k] for k in ("W1", "A1", "RHS2")}
        if has_bias:
            m["b1row"] = shared["b1row"]
            m["b2row"] = shared["b2row"]
        m["xT"] = xT[c]
        pc = dict(per_core[c])
        pc["s0t_bf"] = pc["s0t_bf"].view(ml_dtypes.float8_e4m3fn)
        m.update(pc)
        in_maps.append(m)
    res = run_bass_kernel_spmd(nc, in_maps, core_ids=list(range(cfg.NCORES)),
                               **run_kwargs)
    out = np.concatenate([res.results[c]["out"] for c in range(cfg.NCORES)], 0)
    return out.astype(np.float32), res


# revision 18
# speedup vs baseline: 1.0405x; 1.0019x over previous
"""Trainium2 Bass kernel for 2-layer GAT (nn_GAT_72619307041134).

Strategy (dst-sharded edge parallelism, 8 cores SPMD):
- Nodes sharded into 8 contiguous ranges of 6250; edges sorted by dst and
  sharded by dst range, so each core owns ALL edges of its dst nodes and the
  segment softmax + aggregation need no cross-core reduction.
- Self-loops are NOT in the edge stream; their contribution is folded into
  the window epilogues (per-node dense math), saving ~6% of gather traffic.
- Per layer, a per-node gather table lives in DRAM:
    table1[n] = [al_src (8 bf16) | h1 (128 fp8)] 256B fp8 rows
    table2[n] = [al2_src (1 f32 = 2 slots) | h2 (64 bf16) | pad] 256B rows
  The dst-side attention logits (al_d) are NOT gathered per edge: within a
  128-node dst window they are broadcast to edges with a small PE matmul
  alde[e,h] = sum_j S0T[j,e] * al_d[j,h], where S0T is the transposed
  edge->node indicator, host-baked as exact fp8 0/1 and streamed from DRAM
  on the (otherwise idle) HWDGE path.
- Edges are processed in 128-edge chunks (16 chunks = 1 super-chunk):
  dma_gather fetches the src rows (int16 indices; srcs >= SPLIT gather
  from a shifted view; A/B-pure chunks, A-first in each super-chunk),
  each call round-robined over all 4 SWDGE queues (the drain of these
  random 256-512B HBM reads is the kernel's bottleneck).
  DVE computes ee = exp(leaky_relu(al_s+al_d)) and msg = [ee*h | ee] (bf16);
  an indicator S0[e,j] = (dst_rel[e]==j) turns the per-window segment-sum
  into PE matmuls accumulating in PSUM.  Window epilogue adds the self-loop
  term and divides by the softmax denominator (segment-max shift skipped;
  logits are bounded by ~4).
- Pad edges have dst_rel=-1 (S0/S0T column zero) and gather row 0.
- Between layers one AllGather shares each core's table2 shard.
- Host preprocessing only sorts/pads/permutes integer indices.
"""

from contextlib import ExitStack

import numpy as np

# ---------------------------------------------------------------------------
# config
# ---------------------------------------------------------------------------


class Cfg:
    def __init__(self, N=50000, E=800000, NCORES=8, WIN=128, CHUNK=128, SC=16,
                 SPLIT=32768):
        self.N = N
        self.E = E
        self.NCORES = NCORES
        self.NSH = N // NCORES
        self.WIN = WIN
        self.NWIN = (self.NSH + WIN - 1) // WIN
        self.CHUNK = CHUNK
        self.SC = SC
        self.SPLIT = SPLIT        # src < SPLIT -> table A view, else B view
        self.HEADS = 8
        self.HID = 16
        self.OUT_C = 64
        self.R1 = 256             # bf16 elems/row: [al_s 8 f32 = 16 slots | h1 128 bf16 | pad]
        self.H1OFF = 16
        self.R2 = 128             # bf16 elems/row: [al2_s 2 slots | h2 64 | pad]
        self.H2OFF = 2
        self.MSG1 = 128 + 8
        self.MSG2 = 64 + 1
        self.NQUEUES = 4


FULL = Cfg()

# ---------------------------------------------------------------------------
# host-side edge preprocessing (indices only)
# ---------------------------------------------------------------------------


def prep_edges(edge_index: np.ndarray, cfg: Cfg):
    """Builds the SPMD-common chunk schedule and per-core index arrays.

    meta: cmap (slot -> (window, first, last)), nA (A-chunks per super-chunk,
    A-first slot order), TC, NG.
    per_core[c]: int16 'src_idx' [16, EPC/16] (wrapped: edge i of the
    slot-ordered stream at [i%16, i//16]), f32 'dst_rel' [128, TC]
    (rel[e, slot] for the edge at (slot, partition e), -1 for pads),
    int16 's0t_bits' [128, TC*8] (bit (j, slot, e) = (rel[slot,e]==j),
    packed 16 edges/word along e).
    """
    N, NSH, WIN, CHUNK, SC = cfg.N, cfg.NSH, cfg.WIN, cfg.CHUNK, cfg.SC
    src = edge_index[0].astype(np.int64)
    dst = edge_index[1].astype(np.int64)
    order = np.argsort(dst, kind="stable")
    src = src[order]
    dst = dst[order]
    isA = src < cfg.SPLIT
    core_of = dst // NSH
    w_of = (dst % NSH) // WIN
    cntA = np.zeros((cfg.NCORES, cfg.NWIN), np.int64)
    cntB = np.zeros((cfg.NCORES, cfg.NWIN), np.int64)
    np.add.at(cntA, (core_of[isA], w_of[isA]), 1)
    np.add.at(cntB, (core_of[~isA], w_of[~isA]), 1)
    cpwA = np.ceil(cntA.max(0) / CHUNK).astype(np.int64)
    cpwB = np.ceil(cntB.max(0) / CHUNK).astype(np.int64)
    tc = int((cpwA + cpwB).sum())
    cpwA[-1] += (-tc) % SC
    TC = int((cpwA + cpwB).sum())
    NG = TC // SC

    # global chunk list: window-major, A then B inside a window
    glist = []  # (window, is_A)
    for w in range(cfg.NWIN):
        glist += [(w, True)] * int(cpwA[w]) + [(w, False)] * int(cpwB[w])
    # per-super-chunk stable reorder: A-chunks first
    slot_of = []
    nA = []
    for g in range(NG):
        blk = list(range(g * SC, (g + 1) * SC))
        a = [i for i in blk if glist[i][1]]
        b = [i for i in blk if not glist[i][1]]
        nA.append(len(a))
        slot_of += a + b
    lastslot = {}
    for s, gi in enumerate(slot_of):
        lastslot[glist[gi][0]] = s
    cmap = []
    seen = set()
    inflight = 0
    max_inflight = 0
    for s, gi in enumerate(slot_of):
        w = glist[gi][0]
        first = w not in seen
        seen.add(w)
        last = lastslot[w] == s
        if first:
            inflight += 1
            max_inflight = max(max_inflight, inflight)
        cmap.append((w, first, last))
        if last:
            inflight -= 1

    starts = np.searchsorted(
        dst, (np.arange(0, N, NSH)[:, None] + np.arange(0, NSH, WIN)[None, :]))
    flat_starts = list(starts.ravel()) + [len(dst)]
    per_core = []
    for c in range(cfg.NCORES):
        s_by_g = np.zeros((len(glist), CHUNK), np.int64)
        r_by_g = np.full((len(glist), CHUNK), -1, np.int64)
        gi = 0
        for w in range(cfg.NWIN):
            i = c * cfg.NWIN + w
            s0, s1 = flat_starts[i], flat_starts[i + 1]
            sl = src[s0:s1]
            dl = dst[s0:s1]
            il = isA[s0:s1]
            for grp in (True, False):
                m = il == grp
                se = sl[m]
                de = dl[m]
                nch = int(cpwA[w]) if grp else int(cpwB[w])
                buf_s = np.zeros(nch * CHUNK, np.int64)
                buf_r = np.full(nch * CHUNK, -1, np.int64)
                buf_s[:len(se)] = se - (0 if grp else cfg.SPLIT)
                buf_r[:len(se)] = de - (c * NSH + w * WIN)
                s_by_g[gi:gi + nch] = buf_s.reshape(nch, CHUNK)
                r_by_g[gi:gi + nch] = buf_r.reshape(nch, CHUNK)
                gi += nch
        svals = s_by_g[slot_of]   # [TC, 128] slot-ordered
        rvals = r_by_g[slot_of]

        def wrap(vals):
            st = vals.reshape(-1)   # stream i = slot*128 + p
            n = st.shape[0]
            out = np.zeros((16, n // 16), np.int16)
            idx = np.arange(n)
            out[idx % 16, idx // 16] = st
            return np.tile(out, (8, 1))

        # host-baked transposed one-hot (fp8 e4m3 bit pattern: 1.0 = 0x38)
        ONE = np.uint8(0x38)
        jj = np.arange(128, dtype=np.int64)
        # s0t[j, slot*128+e] = (rvals[slot, e] == j)
        s0t = (rvals[None, :, :] == jj[:, None, None]).astype(np.uint8) * ONE
        per_core.append(dict(
            src_idx=wrap(svals),
            rel_bf=np.ascontiguousarray(rvals.T).astype(np.float32),
            s0t_bf=np.ascontiguousarray(s0t.reshape(128, TC * 128)),
        ))
    meta = dict(cmap=cmap, nA=nA, TC=TC, NG=NG, max_inflight=max_inflight)
    return meta, per_core


def host_tensors(inputs, cfg: Cfg):
    x = np.ascontiguousarray(inputs["x"], np.float32)
    W1 = np.ascontiguousarray(inputs["W1"], np.float32)
    a1s = np.asarray(inputs["a1_src"], np.float32)
    a1d = np.asarray(inputs["a1_dst"], np.float32)
    W2 = np.ascontiguousarray(inputs["W2"], np.float32)
    a2s = np.asarray(inputs["a2_src"], np.float32).reshape(1, -1)
    a2d = np.asarray(inputs["a2_dst"], np.float32).reshape(1, -1)
    b1 = np.asarray(inputs["b1"], np.float32)
    b2 = np.asarray(inputs["b2"], np.float32)
    H, HID = cfg.HEADS, cfg.HID
    A1 = np.zeros((H * HID, 2 * H), np.float32)
    for h in range(H):
        A1[h * HID:(h + 1) * HID, h] = a1s[h]
        A1[h * HID:(h + 1) * HID, H + h] = a1d[h]
    RHS2 = np.concatenate([W2 @ a2s.T, W2 @ a2d.T, W2], 1).astype(np.float32)
    shared = dict(W1=W1, A1=A1, RHS2=RHS2,
                  b1row=b1.reshape(1, -1),
                  b2row=np.concatenate([np.zeros(2, np.float32), b2]).reshape(1, -1))
    xT = [np.ascontiguousarray(x[c * cfg.NSH:(c + 1) * cfg.NSH].T)
          for c in range(cfg.NCORES)]
    has_bias = bool(np.any(b1) or np.any(b2))
    return shared, xT, has_bias


# ---------------------------------------------------------------------------
# device kernel emission
# ---------------------------------------------------------------------------


def _ap(base, free_dims, extra_off=0):
    """Replace the free dims of a [P, ...] AP (keep partition dim)."""
    import concourse.bass as bass

    return bass.AP(base.tensor, base.offset + extra_off,
                   [list(base.ap[0])] + [list(d) for d in free_dims])


def emit_gat(tc, out_ap, ins, meta, cfg: Cfg, has_bias=False):
    import concourse.bass as bass  # noqa: F401
    from concourse import mybir

    nc = tc.nc
    f32 = mybir.dt.float32
    bf16 = mybir.dt.bfloat16
    fp8 = mybir.dt.float8e4
    i16 = mybir.dt.int16
    i32 = mybir.dt.int32
    AF = mybir.ActivationFunctionType
    OP = mybir.AluOpType
    N, NSH, WIN, NWIN, SC = cfg.N, cfg.NSH, cfg.WIN, cfg.NWIN, cfg.SC
    TC, NG = meta["TC"], meta["NG"]
    cmap, nA = meta["cmap"], meta["nA"]
    NQ = getattr(cfg, "NQUEUES", 1)
    R1, R2 = cfg.R1, cfg.R2
    M1, M2 = cfg.MSG1, cfg.MSG2
    NIW = TC * cfg.CHUNK // 16

    ctx = ExitStack()
    with ctx:
        dram = ctx.enter_context(tc.tile_pool(name="dram", bufs=1, space="DRAM"))
        consts = ctx.enter_context(tc.tile_pool(name="consts", bufs=1))

        t1shard = dram.tile([NSH, R1], bf16)
        t1full = dram.tile([N, R1], bf16, addr_space="Shared")
        t2shard = dram.tile([NSH, R2], bf16)
        t2full = dram.tile([N, R2], bf16, addr_space="Shared")

        # ------- constants into SBUF -------
        W1_sb = consts.tile([128, 128], f32)
        A1_sb = consts.tile([128, 16], f32)
        RHS2_sb = consts.tile([128, 66], f32)
        nc.sync.dma_start(W1_sb[:], ins["W1"][:])
        nc.sync.dma_start(A1_sb[:], ins["A1"][:])
        nc.sync.dma_start(RHS2_sb[:], ins["RHS2"][:])
        src_sb = consts.tile([128, NIW], i16)
        nc.sync.dma_start(src_sb[:], ins["src_idx"][:])
        relf_sb = consts.tile([128, TC], f32)
        rel_sb = consts.tile([128, TC], bf16)
        nc.sync.dma_start(relf_sb[:], ins["rel_bf"][:])
        if has_bias:
            b1_sb = consts.tile([1, 128], f32)
            b2_sb = consts.tile([1, 66], f32)
            ones_sb = consts.tile([1, 128], f32)
            nc.sync.dma_start(b1_sb[:], ins["b1row"][:])
            nc.sync.dma_start(b2_sb[:], ins["b2row"][:])
            nc.gpsimd.memset(ones_sb[:], 1.0)

        # identity + iota + bitmask
        iota_i = consts.tile([128, 128], i32)
        icol = consts.tile([128, 1], i32)
        id_sb = consts.tile([128, 128], f32)
        iota_f = consts.tile([128, 128], f32)
        icol_f = consts.tile([128, 1], f32)
        nc.gpsimd.iota(iota_i[:], pattern=[[1, 128]], base=0, channel_multiplier=0)
        nc.gpsimd.iota(icol[:], pattern=[[1, 1]], base=0, channel_multiplier=1)
        nc.vector.tensor_copy(iota_f[:], iota_i[:])
        nc.vector.tensor_copy(icol_f[:], icol[:])
        nc.vector.tensor_scalar(id_sb[:], iota_f[:], icol_f[:], None, OP.is_equal)
        iota_bf = consts.tile([128, 128], bf16)
        nc.vector.tensor_copy(iota_bf[:], iota_f[:])
        nc.vector.tensor_copy(rel_sb[:], relf_sb[:])

        # persistent per-node state (per window layout)
        als_sb = consts.tile([128, NWIN * 16], f32)     # [al_s 8 | al_d 8]
        aldw_bf = consts.tile([128, NWIN * 8], bf16)    # al_d (L1)
        al2_sb = consts.tile([128, NWIN * 2], f32)      # [al2_s, al2_d]
        al2_bf = consts.tile([128, NWIN], bf16)         # al2_d
        h1w_sb = consts.tile([128, NWIN * 128], bf16)   # h1 rows per window
        h2w_sb = consts.tile([128, NWIN * 64], bf16)    # h2 rows per window
        nc.vector.memset(aldw_bf[:], 0.0)
        nc.vector.memset(al2_bf[:], 0.0)

        # ------- stage A: h1, al, table1 -------
        with tc.tile_pool(name="stageA", bufs=1) as sa, \
             tc.tile_pool(name="stageApsum", bufs=2, space="PSUM") as sap, \
             tc.tile_pool(name="rows", bufs=3) as rows:
            xT_sb = sa.tile([128, NSH], f32)
            nc.sync.dma_start(xT_sb[:], ins["xT"][:])
            h1T_sb = sa.tile([128, NSH], f32)
            al_sb = sa.tile([16, NSH], f32)
            nt = (NSH + 511) // 512
            for j in range(nt):
                w0 = j * 512
                w1 = min(NSH, w0 + 512)
                ph = sap.tile([128, 512], f32, tag="ph")
                nc.tensor.matmul(ph[:, : w1 - w0], W1_sb[:], xT_sb[:, w0:w1],
                                 start=True, stop=True)
                nc.vector.tensor_copy(h1T_sb[:, w0:w1], ph[:, : w1 - w0])
            for j in range(nt):
                w0 = j * 512
                w1 = min(NSH, w0 + 512)
                pa = sap.tile([16, 512], f32, tag="pa")
                nc.tensor.matmul(pa[:, : w1 - w0], A1_sb[:], h1T_sb[:, w0:w1],
                                 start=True, stop=True)
                nc.vector.tensor_copy(al_sb[:, w0:w1], pa[:, : w1 - w0])

            for w in range(NWIN):
                w0 = w * WIN
                wn = min(WIN, NSH - w0)
                hp = sap.tile([128, 128], f32, tag="hp")
                if has_bias:
                    nc.tensor.matmul(hp[:wn, :], xT_sb[:, w0:w0 + wn], W1_sb[:],
                                     start=True, stop=False)
                    nc.tensor.matmul(hp[:wn, :], ones_sb[0:1, :wn], b1_sb[:],
                                     start=False, stop=True)
                else:
                    nc.tensor.matmul(hp[:wn, :], xT_sb[:, w0:w0 + wn], W1_sb[:],
                                     start=True, stop=True)
                at = sap.tile([128, 16], f32, tag="at")
                nc.tensor.transpose(at[:wn, :], al_sb[:, w0:w0 + wn], id_sb[:16, :16])
                rowt = rows.tile([128, R1], bf16, tag="rowt")
                nc.vector.tensor_copy(rowt[:wn, 0:16].bitcast(f32), at[:wn, 0:8])
                nc.scalar.copy(rowt[:wn, 16:144], hp[:wn, :])
                nc.vector.memset(rowt[:wn, 144:R1], 0.0)
                nc.sync.dma_start(t1shard.opt()[w0:w0 + wn, :], rowt[:wn, :])
                nc.vector.tensor_copy(als_sb[:wn, w * 16:(w + 1) * 16], at[:wn, :])
                nc.vector.tensor_copy(aldw_bf[:wn, w * 8:(w + 1) * 8],
                                      at[:wn, 8:16])
                nc.vector.tensor_copy(h1w_sb[:wn, w * 128:(w + 1) * 128],
                                      hp[:wn, :])

        from concourse import library_config

        nc.gpsimd.load_library(library_config.mlp)

        nc.gpsimd.collective_compute(
            "AllGather", mybir.AluOpType.bypass,
            replica_groups=[list(range(cfg.NCORES))],
            ins=[t1shard.opt()], outs=[t1full.opt()],
        )

        # ------- edge layers -------
        def edge_layer(table_full, row, rdt, alst, hoff, nh, chper, msgc,
                       alw_bf, epilogue):
            with tc.tile_pool(name="edges", bufs=8) as epool, \
                 tc.tile_pool(name="emsg", bufs=4) as mp, \
                 tc.tile_pool(name="epsum", bufs=meta["max_inflight"] + 1,
                              space="PSUM") as pp, \
                 tc.tile_pool(name="aldpsum", bufs=2, space="PSUM") as pae, \
                 tc.tile_pool(name="esmall", bufs=4) as spool:
                pw_by_w = {}
                qctr = [0]
                for g in range(NG):
                    na = nA[g]
                    gb = epool.tile([128, SC * row], rdt, tag="gb")
                    gb3 = gb[:].rearrange("p (k e) -> p k e", k=SC)
                    c0 = g * SC * 8
                    for grp in range(2):
                        nch = na if grp == 0 else SC - na
                        if nch == 0:
                            continue
                        k0 = 0 if grp == 0 else na
                        tbl = (table_full.opt() if grp == 0
                               else table_full.opt()[cfg.SPLIT:N, :])
                        nh1 = (nch + 1) // 2
                        for h0, hn in ((0, nh1), (nh1, nch - nh1)):
                            if hn == 0:
                                continue
                            ksl = slice(k0 + h0, k0 + h0 + hn)
                            csl = slice(c0 + (k0 + h0) * 8,
                                        c0 + (k0 + h0 + hn) * 8)
                            nc.gpsimd.dma_gather(
                                gb3[:, ksl, :], tbl, src_sb[:, csl],
                                num_idxs=hn * 128, num_idxs_reg=hn * 128,
                                elem_size=row, single_packet=False,
                                queue_num=qctr[0] % NQ,
                            )
                            qctr[0] += 1
                    # S0T [j, e] streamed from DRAM (host-baked)
                    s0t = epool.tile([128, SC * 128], fp8, tag="s0t")
                    nc.sync.dma_start(
                        s0t[:], ins["s0t_bf"][:, g * SC * 128:(g + 1) * SC * 128])
                    # al_d per edge via PE: alde[e, h] = sum_j S0T[j,e] ald[j,h]
                    pa = pae.tile([128, SC * nh], f32, tag="pa", name="pa")
                    for k in range(SC):
                        w = cmap[g * SC + k][0]
                        nc.tensor.matmul(
                            pa[:, k * nh:(k + 1) * nh],
                            s0t[:, k * 128:(k + 1) * 128],
                            alw_bf[:, w * nh:(w + 1) * nh],
                            start=True, stop=True)
                    lg = spool.tile([128, SC * nh], f32, tag="lg")
                    nc.vector.tensor_tensor(
                        _ap(lg[:], [[nh, SC], [1, nh]]),
                        _ap(gb[:].bitcast(f32), [[alst, SC], [1, nh]]),
                        _ap(pa[:], [[nh, SC], [1, nh]]),
                        OP.add,
                    )
                    lr = spool.tile([128, SC * nh], f32, tag="lr")
                    nc.vector.scalar_tensor_tensor(
                        lr[:], lg[:], 0.2, lg[:], OP.mult, OP.max)
                    ee = spool.tile([128, SC * nh], bf16, tag="ee")
                    nc.scalar.activation(ee[:], lr[:], AF.Exp)
                    msg = mp.tile([128, SC * msgc], bf16, tag="msg")
                    nc.vector.tensor_tensor(
                        _ap(msg[:], [[msgc, SC], [chper, nh], [1, chper]]),
                        _ap(gb[:], [[row, SC], [chper, nh], [1, chper]], hoff),
                        _ap(ee[:], [[nh, SC], [1, nh], [0, chper]]),
                        OP.mult,
                    )
                    nc.scalar.copy(
                        _ap(msg[:], [[msgc, SC], [1, nh]], msgc - nh),
                        _ap(ee[:], [[nh, SC], [1, nh]]),
                    )
                    s0 = epool.tile([128, SC * 128], fp8, tag="s0")
                    nc.vector.tensor_tensor(
                        _ap(s0[:], [[128, SC], [1, 128]]),
                        _ap(iota_bf[:], [[0, SC], [1, 128]]),
                        _ap(rel_sb[:, g * SC:(g + 1) * SC], [[1, SC], [0, 128]]),
                        OP.is_equal,
                    )
                    for k in range(SC):
                        kk = g * SC + k
                        w, first, last = cmap[kk]
                        if first:
                            pw_by_w[w] = pp.tile([128, msgc], f32, tag="pw", name="pw")
                        pw = pw_by_w[w]
                        nc.tensor.matmul(
                            pw[:], s0[:, k * 128:(k + 1) * 128],
                            msg[:, k * msgc:(k + 1) * msgc],
                            start=first, stop=last,
                        )
                        if last:
                            epilogue(w, pw_by_w.pop(w))

        # ---- L1 ----
        with tc.tile_pool(name="epi1", bufs=2) as hq, \
             tc.tile_pool(name="epi1p", bufs=1, space="PSUM") as hpp:
            def epi1(w, pw):
                w0 = w * WIN
                wn = min(WIN, NSH - w0)
                # self-loop term
                lgs = hq.tile([128, 8], f32, tag="lgs")
                nc.vector.tensor_tensor(lgs[:], als_sb[:, w * 16:w * 16 + 8],
                                        als_sb[:, w * 16 + 8:w * 16 + 16], OP.add)
                ees = hq.tile([128, 8], f32, tag="ees")
                nc.vector.scalar_tensor_tensor(
                    ees[:], lgs[:], 0.2, lgs[:], OP.mult, OP.max)
                nc.scalar.activation(ees[:], ees[:], AF.Exp)
                dn = hq.tile([128, 8], f32, tag="dn")
                nc.vector.scalar_tensor_tensor(
                    dn[:], pw[:, 128:136], 1e-16, ees[:], OP.add, OP.add)
                rcp = hq.tile([128, 8], f32, tag="rcp")
                nc.vector.reciprocal(rcp[:], dn[:])
                uu = hq.tile([128, 128], f32, tag="uu")
                nc.vector.tensor_tensor(
                    _ap(uu[:], [[16, 8], [1, 16]]),
                    _ap(h1w_sb[:, w * 128:(w + 1) * 128], [[16, 8], [1, 16]]),
                    _ap(ees[:], [[1, 8], [0, 16]]),
                    OP.mult,
                )
                nc.vector.tensor_tensor(uu[:], uu[:], pw[:, 0:128], OP.add)
                hb = hq.tile([128, 128], f32, tag="hb")
                nc.vector.tensor_tensor(
                    _ap(hb[:], [[16, 8], [1, 16]]),
                    _ap(uu[:], [[16, 8], [1, 16]]),
                    _ap(rcp[:], [[1, 8], [0, 16]]),
                    OP.mult,
                )
                nc.scalar.activation(hb[:], hb[:], AF.Relu)
                tp = hpp.tile([128, 128], f32, tag="tp")
                nc.tensor.transpose(tp[:], hb[:], id_sb[:])
                tH = hq.tile([128, 128], f32, tag="tH")
                nc.vector.tensor_copy(tH[:], tp[:])
                p2 = hpp.tile([128, 66], f32, tag="p2")
                if has_bias:
                    nc.tensor.matmul(p2[:], tH[:], RHS2_sb[:], start=True, stop=False)
                    nc.tensor.matmul(p2[:], ones_sb[0:1, :128], b2_sb[:],
                                     start=False, stop=True)
                else:
                    nc.tensor.matmul(p2[:], tH[:], RHS2_sb[:], start=True, stop=True)
                t2b = hq.tile([128, R2], bf16, tag="t2b")
                nc.vector.tensor_copy(t2b[:wn, 0:2].bitcast(f32), p2[:wn, 0:1])
                nc.scalar.copy(t2b[:wn, 2:66], p2[:wn, 2:66])
                nc.vector.memset(t2b[:wn, 66:R2], 0.0)
                nc.sync.dma_start(t2shard.opt()[w0:w0 + wn, :], t2b[:wn, :])
                nc.vector.tensor_copy(al2_sb[:wn, w * 2:(w + 1) * 2], p2[:wn, 0:2])
                nc.vector.tensor_copy(al2_bf[:wn, w:w + 1], p2[:wn, 1:2])
                nc.vector.tensor_copy(h2w_sb[:wn, w * 64:(w + 1) * 64],
                                      p2[:wn, 2:66])

            edge_layer(t1full, cfg.R1, mybir.dt.bfloat16, 128, cfg.H1OFF,
                       8, 16, M1, aldw_bf, epi1)

        nc.gpsimd.collective_compute(
            "AllGather", mybir.AluOpType.bypass,
            replica_groups=[list(range(cfg.NCORES))],
            ins=[t2shard.opt()], outs=[t2full.opt()],
        )

        # ---- L2 ----
        with tc.tile_pool(name="epi2", bufs=2) as oq:
            def epi2(w, pw):
                w0 = w * WIN
                wn = min(WIN, NSH - w0)
                lg2 = oq.tile([128, 1], f32, tag="lg2")
                nc.vector.tensor_tensor(lg2[:], al2_sb[:, w * 2:w * 2 + 1],
                                        al2_sb[:, w * 2 + 1:w * 2 + 2], OP.add)
                ee2 = oq.tile([128, 1], f32, tag="ee2")
                nc.vector.scalar_tensor_tensor(
                    ee2[:], lg2[:], 0.2, lg2[:], OP.mult, OP.max)
                nc.scalar.activation(ee2[:], ee2[:], AF.Exp)
                dn2 = oq.tile([128, 1], f32, tag="dn2")
                nc.vector.scalar_tensor_tensor(
                    dn2[:], pw[:, 64:65], 1e-16, ee2[:], OP.add, OP.add)
                rcp2 = oq.tile([128, 1], f32, tag="rcp2")
                nc.vector.reciprocal(rcp2[:], dn2[:])
                ms2 = oq.tile([128, 64], f32, tag="ms2")
                nc.vector.tensor_tensor(
                    ms2[:], h2w_sb[:, w * 64:(w + 1) * 64],
                    _ap(ee2[:], [[0, 64]]), OP.mult)
                nc.vector.tensor_tensor(ms2[:], ms2[:], pw[:, 0:64], OP.add)
                ob = oq.tile([128, 64], f32, tag="ob")
                nc.vector.tensor_tensor(ob[:], ms2[:], _ap(rcp2[:], [[0, 64]]),
                                        OP.mult)
                nc.sync.dma_start(out_ap[w0:w0 + wn, :], ob[:wn, :])

            edge_layer(t2full, cfg.R2, mybir.dt.bfloat16, 64, cfg.H2OFF,
                       1, 64, M2, al2_bf, epi2)


# ---------------------------------------------------------------------------
# SPMD build + run
# ---------------------------------------------------------------------------

_CACHE = {}


def _build(meta, cfg: Cfg, has_bias: bool):
    key = (tuple(meta["cmap"]), tuple(meta["nA"]), cfg.N, cfg.NCORES, has_bias)
    if key in _CACHE:
        return _CACHE[key]
    import concourse.tile as tile
    from concourse import bacc, mybir

    f32 = mybir.dt.float32
    i16 = mybir.dt.int16
    TC = meta["TC"]
    NIW = TC * cfg.CHUNK // 16
    nc = bacc.Bacc("TRN2", target_bir_lowering=False, debug=False,
                   num_devices=cfg.NCORES,
                   num_swdge_queues=getattr(cfg, "NQUEUES", 1))
    ins = {}

    def di(name, shape, dt=f32):
        ins[name] = nc.dram_tensor(name, shape, dt, kind="ExternalInput").ap()

    di("xT", [128, cfg.NSH])
    di("W1", [128, 128])
    di("A1", [128, 16])
    di("RHS2", [128, 66])
    di("src_idx", [128, NIW], i16)
    di("rel_bf", [128, TC])
    di("s0t_bf", [128, TC * 128], mybir.dt.float8e4)
    if has_bias:
        di("b1row", [1, 128])
        di("b2row", [1, 66])
    out = nc.dram_tensor("out", [cfg.NSH, cfg.OUT_C], f32, kind="ExternalOutput").ap()

    with tile.TileContext(nc) as tc:
        emit_gat(tc, out, ins, meta, cfg, has_bias)
    nc.compile()
    _CACHE[key] = nc
    return nc


def kernel(**inputs) -> np.ndarray:
    out, _ = _run(inputs)
    return out


def _run(inputs, **run_kwargs):
    cfg = FULL
    inputs = {k: np.asarray(v) for k, v in inputs.items()}
    edge_index = inputs["edge_index"].astype(np.int64)
    meta, per_core = prep_edges(edge_index, cfg)
    shared, xT, has_bias = host_tensors(inputs, cfg)
    nc = _build(meta, cfg, has_bias)

    from concourse.bass_utils import run_bass_kernel_spmd

    import ml_dtypes

    in_maps = []
    for c in range(cfg.NCORES):
        m = {k: shared[k] for k in ("W1", "A1", "RHS2")}
        if has_bias:
            m["b1row"] = shared["b1row"]
            m["b2row"] = shared["b2row"]
        m["xT"] = xT[c]
        pc = dict(per_core[c])
        pc["s0t_bf"] = pc["s0t_bf"].view(ml_dtypes.float8_e4m3fn)
        m.update(pc)
        in_maps.append(m)
    res = run_bass_kernel_spmd(nc, in_maps, core_ids=list(range(cfg.NCORES)),
                               **run_kwargs)
    out = np.concatenate([res.results[c]["out"] for c in range(cfg.NCORES)], 0)
    return out.astype(np.float32), res
